# revision 22
# baseline (speedup 1.0000x reference)
"""Trainium2 Bass kernel for nn_EntityMentionAggregation.

Reference computation (per batch b, M=N=16 mentions, H=768):
  self-attn over head mentions, cross-attn head->tail, sigmoid-gated fusion,
  mask-softmax pooling over mentions -> out [B, H].

Algebraic restructuring (exact, given the zero biases produced by
setup_inputs; nonzero projection biases fall back to numpy):
  s_scores = scale * head @ (Wsq^T Wsk) @ head^T          (A_s folded)
  c_scores = scale * head @ (Wcq^T Wck) @ tail^T          (A_c folded)
  out      = hpool @ Wsv^T + tpool
    hpool  = ws_s^T-weighted sum of head rows, ws_s = s_w^T (mw*gate/den_s)
    tpool  = ws_c^T-weighted sum of tail rows
  gate     = sigmoid(s_w@(head@u) + c_w@(tail@w2) + C0), u = Wsv^T Wg1
so the V projection runs on pooled vectors (16x fewer rows) and
self_out/cross_out are never materialized.

Precision split: the score path (big GEMM + packed per-tile attention
matmuls + gate dot-products) runs in fp8 e4m3 with DoubleRow perf mode
(2 k-tiles of 128 per matmul at 0.5 cyc/row). On the value path the TAIL
pools stay fp16 (tpool lands raw in the output; fp8 there costs 3.5e-2
rel err) while the HEAD pools reuse the fp8 copy (hpool's quantization
noise washes through the Wsv^T projection; 1.35e-2 total, under the
2e-2 gate), which drops the fp16 head load entirely. The fp8 operands are
produced by a second SWDGE cast-load (f32->fp8) and transposed to
feature-major via the SBUF xbar with PAIRS of fp8 values packed in one
uint16 element; the resulting [feat-pair partition, 2, row] layout is
exactly DoubleRow's expected [K,2,N] k-tile shape (logical feature
f = 256c + 2p + i).  The folded A matrix is stored column-permuted
(per 256-block: even columns then odd columns) so the big GEMM's PSUM
partitions line up with the same pairing when its output chunks are used
as score-matmul weights.

Gate path: e = exp(scores/S) is transposed on the PE (identity matmul) so
den = e^T @ (-S*ones) and gs = e^T @ (head@u) become 1-column matmuls,
removing the partition-broadcast DRAM round-trip of hv entirely. The
sigmoid is evaluated as 1/(1+exp(-garg)) so the ACT engine only ever
needs the Exp table (Sigmoid lives in a different act-table set; each
switch would cost a 1.3us table reload); the sign flips ride host
constants (onesc=-S, mw=-S*mw, c0=-C0) and cancel in a_s/a_c. u and w2
are stored as fp8 hi+lo residual pairs - the gate dot-products were the
precision-dominant path (rel err 1.26e-2 -> 6.7e-3 on HW).
Scores carry a factor S=32 (folded into A) so the fp8 tensors sit inside
e4m3's dynamic range (max 240); exp applies scale 1/S.

Layout: batch is sharded 8 ways (512 batches/core); rows are processed in
tiles of 128 = 8 batches x 16 mentions. Cross-batch blocks are masked to
-inf via a rank-9 constant matmul and the per-tile key-mask via a rank-1
matmul, so softmax zeroes them exactly and the packed attention matrix is
block-diagonal -- which makes the pooling contractions plain matmuls
against block-diagonal weight columns built with a onehot multiply.
"""

import numpy as np
import ml_dtypes
import bass_rust
import concourse.bass as bass
import concourse.mybir as mybir
import concourse.tile as tile
from concourse.bass import ts
from concourse.bass_utils import run_bass_kernel_spmd

H = 768
B, M, N = 4096, 16, 16
NEG = -65504.0
P = 128
NCORES = 8
BC = B // NCORES          # batches per core = 512
ROWS = BC * M             # rows per core = 8192
TILES = ROWS // P         # 64 tiles (8 batches each)
ST = 4                    # tiles per supertile (GEMM moving N = 512)
NSUP = TILES // ST        # 16 supertiles
SN = ST * P               # 512 rows per supertile
GN = 512                  # GEMM moving width per PSUM pass (one bank)
KC = H // P               # 6 contraction chunks (128 each)
KC2 = H // (2 * P)        # 3 DoubleRow chunk pairs (256 each)
FO = 2 * H // P           # 12 score-feature chunks (A_s | A_c)
ACOLS = 2 * H + 1         # 1537: A_s | A_c | u
APAD = 1552               # ACOLS padded so the DoubleRow pair step is 16B-aligned
RPB = ROWS // M           # 512 pooled rows (batches) per core
S = 32.0                  # fp8 dynamic-range scale folded into A/u/w2/mw

F8 = mybir.dt.float8e4
F16 = mybir.dt.float16
F32 = mybir.dt.float32
U16 = mybir.dt.uint16
DR = mybir.MatmulPerfMode.DoubleRow
NP8 = ml_dtypes.float8_e4m3


def _split_sync_waits(nc):
    """Walrus caps sync waits per instruction (1 is the only universally
    accepted count in this toolchain). Hoist excess waits onto preceding
    single-wait EventSemaphore carriers on the same engine."""
    for f in nc.m.functions:
        for bb in f.blocks:
            il = bb.instructions
            new_il = []
            changed = False
            for inst in il:
                si = inst.sync_info
                if si is not None and len(si.on_wait) > 1:
                    waits = list(si.on_wait)
                    k = 0
                    while len(waits) > 1:
                        w, waits = waits[0], waits[1:]
                        d = bass_rust.InstEventSemaphore(
                            name=f"{inst.name}-wsplit{k}", ins=[], outs=[])
                        d.engine = inst.engine
                        d.sync_info = bass_rust.SyncInfo(on_wait=[w], on_update=[])
                        new_il.append(d)
                        k += 1
                        changed = True
                    inst.sync_info = bass_rust.SyncInfo(
                        on_wait=waits, on_update=list(si.on_update))
                new_il.append(inst)
            if changed:
                bb.instructions = new_il


def _build_nc(split=True):
    nc = bass.Bass(target_bir_lowering=False)

    head_d = nc.dram_tensor("head", [ROWS, H], F32, kind="ExternalInput")
    tail8_d = nc.dram_tensor("tail8", [ROWS, H], F8, kind="ExternalInput")
    taillo_d = nc.dram_tensor("taillo", [ROWS, H], F32, kind="ExternalInput")
    acat_d = nc.dram_tensor("acat", [KC2, P, 2, APAD], F8, kind="ExternalInput")
    w2_d = nc.dram_tensor("w2c", [KC2, P, 2, 2], F8, kind="ExternalInput")
    wsvT_d = nc.dram_tensor("wsvT", [KC, P, H], F16, kind="ExternalInput")
    c10_d = nc.dram_tensor("c10", [10, P], F16, kind="ExternalInput")
    onesc_d = nc.dram_tensor("onesc", [P, 1], F16, kind="ExternalInput")
    onehot_d = nc.dram_tensor("onehot", [P, ST * 8], F16, kind="ExternalInput")
    vs_d = nc.dram_tensor("vs", [10, TILES * P], F16, kind="ExternalInput")
    vc_d = nc.dram_tensor("vc", [10, TILES * P], F16, kind="ExternalInput")
    mw_d = nc.dram_tensor("mw", [P, TILES], F32, kind="ExternalInput")
    identh_d = nc.dram_tensor("identh", [P, P], F16, kind="ExternalInput")
    c0_d = nc.dram_tensor("c0", [P, 1], F32, kind="ExternalInput")
    out_d = nc.dram_tensor("out", [BC, H], F32, kind="ExternalOutput")

    with tile.TileContext(nc) as tc:
        _emit(nc, tc, head_d, tail8_d, taillo_d, acat_d, w2_d, wsvT_d, c10_d,
              onesc_d, onehot_d, vs_d, vc_d, mw_d,
              identh_d, c0_d, out_d)
    if split:
        _split_sync_waits(nc)
    return nc


def _emit(nc, tc, head_d, tail8_d, taillo_d, acat_d, w2_d, wsvT_d, c10_d,
          onesc_d, onehot_d, vs_d, vc_d, mw_d,
          identh_d, c0_d, out_d):
    from contextlib import ExitStack
    Exp = mybir.ActivationFunctionType.Exp
    Sig = mybir.ActivationFunctionType.Sigmoid
    Ident = mybir.ActivationFunctionType.Identity
    mult = mybir.AluOpType.mult
    ctx = ExitStack()
    with ctx:
        const = ctx.enter_context(tc.tile_pool(name="const", bufs=1))
        sup = ctx.enter_context(tc.tile_pool(name="sup", bufs=2))
        pt = ctx.enter_context(tc.tile_pool(name="pt", bufs=8))
        acc = ctx.enter_context(tc.tile_pool(name="acc", bufs=1))
        psg = ctx.enter_context(tc.tile_pool(name="psg", bufs=2, space="PSUM"))
        pss = ctx.enter_context(tc.tile_pool(name="pss", bufs=2, space="PSUM"))
        psw = ctx.enter_context(tc.tile_pool(name="psw", bufs=2, space="PSUM"))

        # ---- constants ----
        acat8 = const.tile([P, KC2, 2, APAD], F8)
        nc.sync.dma_start(out=acat8[:], in_=acat_d.rearrange("c p i m -> p c i m"))
        w2c8 = const.tile([P, KC2, 2, 2], F8)
        nc.sync.dma_start(out=w2c8[:], in_=w2_d.rearrange("c p i m -> p c i m"))
        c10 = const.tile([10, P], F16)
        nc.sync.dma_start(out=c10[:], in_=c10_d[:, :])
        onesc = const.tile([P, 1], F16)
        nc.sync.dma_start(out=onesc[:], in_=onesc_d[:, :])
        onehot4 = const.tile([P, ST, 8], F16)
        nc.sync.dma_start(out=onehot4[:], in_=onehot_d.rearrange(
            "p (t e) -> p t e", e=8))
        vs_all = const.tile([10, TILES * P], F16)
        nc.sync.dma_start(out=vs_all[:], in_=vs_d[:, :])
        vc_all = const.tile([10, TILES * P], F16)
        nc.sync.dma_start(out=vc_all[:], in_=vc_d[:, :])

        # ---- per-core accumulators ----
        hp_all = acc.tile([P, KC, RPB], F16)   # pooled head, feature-major
        tp_all = acc.tile([P, KC, RPB], F16)   # pooled tail, feature-major

        # loads are batched per PAIR of supertiles: the SWDGE descriptor-gen
        # time on the Pool engine is ~1-2.4us per instruction regardless of
        # size, and 4 cast-loads/supertile made Pool the DMA-issue serializer
        SG = 2 * ST
        GSUP = SG // ST
        head_r = head_d.rearrange("(g t p) h -> g p t h", t=SG, p=P)
        tail8_r = tail8_d.rearrange("(g t p) h -> g p t h", t=SG, p=P)
        taillo_r = taillo_d.rearrange("(g t p) h -> g p t h", t=SG, p=P)

        loaded16 = {}
        loaded8 = {}

        def emit_loads16(g):
            # fp8 LO residual of tail (host-split): tail = t8 + t8lo to
            # ~2^-8 relative, which the TAIL value pools need (tpool lands
            # raw in the output; single fp8 there costs 3.5e-2 rel err).
            # The HEAD value pools read the fp8 h8 copy alone -- hpool's
            # quantization noise washes through the Wsv^T projection.
            t8lo = sup.tile([P, SG, H], F8, tag="t8lo", name=f"t8lo_{g}")
            nc.gpsimd.dma_start(out=t8lo[:], in_=taillo_r[g])
            loaded16[g] = t8lo

        def emit_loads8(g):
            # fp8 copies feed the score-side GEMMs; h8 cast from the f32
            # rows by SWDGE, t8 pre-cast on host (so the lo residual pairs
            # exactly)
            h8 = sup.tile([P, SG, H], F8, tag="h8", name=f"h8_{g}", bufs=3)
            t8 = sup.tile([P, SG, H], F8, tag="t8", name=f"t8_{g}")
            nc.gpsimd.dma_start(out=h8[:], in_=head_r[g])
            nc.gpsimd.dma_start(out=t8[:], in_=tail8_r[g])
            loaded8[g] = (h8, t8)

        transposed = {}

        def emit_transpose(s):
            # xbar transpose to feature-major with fp8 PAIRS packed in uint16
            h8, t8 = loaded8[s // GSUP]
            off = ST * (s % GSUP)
            hT = sup.tile([P, KC2, SN], U16, tag="hT", name=f"hT{s}", bufs=3)
            tT = sup.tile([P, KC2, SN], U16, tag="tT", name=f"tT{s}", bufs=3)
            for t in range(ST):
                nc.sync.dma_start_transpose(hT[:, :, ts(t, P)],
                                            h8[:, off + t, :].bitcast(U16))
                nc.sync.dma_start_transpose(tT[:, :, ts(t, P)],
                                            t8[:, off + t, :].bitcast(U16))
            transposed[s] = (hT, tT)

        def f8v(tT_, cc):
            # DoubleRow moving view of a pair-packed chunk: [K=128, 2, n]
            return tT_[:, cc, :].bitcast(F8).rearrange(
                "p (n two) -> p two n", two=2)

        out_sb = acc.tile([P, BC // P, H], F32)
        out_r = out_d.rearrange("(r p) h -> p r h", p=P)

        def emit_final(half):
            # out[batch, :] = hpool @ Wsv^T + tpool, computed ROW-major:
            # hp_all chunks are the (free) stationary with batches on the
            # free axis, wsvT moving; tpool is transpose-accumulated into
            # the same PSUM group via an identity moving operand. One batched
            # ACT copy per PSUM bank evacuates, then DMA-store.
            for r in range(half * (BC // P // 2), (half + 1) * (BC // P // 2)):
                pA = psg.tile([P, GN], F32, tag="pg", name=f"pfA{r}")
                pB = psg.tile([P, GN], F32, tag="pg", name=f"pfB{r}")
                for j in range(KC):
                    tgt = pA[:, ts(j, P)] if j < 4 else pB[:, ts(j - 4, P)]
                    for c in range(KC):
                        nc.tensor.matmul(tgt, hp_all[:, c, ts(r, P)],
                                         wsvT[:, c, ts(j, P)],
                                         start=(c == 0), stop=False)
                    nc.tensor.matmul(tgt, tp_all[:, j, ts(r, P)], identh[:],
                                     start=False, stop=True)
                nc.scalar.copy(out_sb[:, r, 0:GN], pA[:])
                nc.scalar.copy(out_sb[:, r, GN:H], pB[:, :H - GN])
                nc.sync.dma_start(out=out_r[:, r, :], in_=out_sb[:, r, :])

        emit_loads16(0)
        emit_loads8(0)
        emit_transpose(0)
        wsvT = const.tile([P, KC, H], F16)
        nc.sync.dma_start(out=wsvT[:], in_=wsvT_d.rearrange("c p m -> p c m"))
        mw_all = const.tile([P, TILES], F32)
        nc.sync.dma_start(out=mw_all[:], in_=mw_d[:, :])
        identh = const.tile([P, P], F16)
        nc.sync.dma_start(out=identh[:], in_=identh_d[:, :])
        c0 = const.tile([P, 1], F32)
        nc.sync.dma_start(out=c0[:], in_=c0_d[:, :])
        hA8s = {}

        def emit_gemm(s):
            # -- big GEMM: hA = head @ [A_s | A_c], feature-major, fp8 DR --
            hT, tT = transposed[s]
            hA8 = sup.tile([P, FO, SN], F8, tag="hA8", name=f"hA8_{s}")
            for j in range(FO):
                for hh in range(SN // GN):
                    pg = psg.tile([P, GN], F32, tag="pg")
                    for cc in range(KC2):
                        nc.tensor.matmul(pg[:], acat8[:, cc, :, ts(j, P)],
                                         f8v(hT, cc)[:, :, ts(hh, GN)],
                                         start=(cc == 0),
                                         stop=(cc == KC2 - 1), perf_mode=DR)
                    if j < 9:
                        nc.scalar.copy(hA8[:, j, ts(hh, GN)], pg[:])
                    else:
                        nc.vector.tensor_copy(hA8[:, j, ts(hh, GN)], pg[:])
            hA8s[s] = hA8

        def emit_tiles(s_idx):
            t8log = loaded16[s_idx // GSUP]
            h8g, t8g = loaded8[s_idx // GSUP]
            voff = ST * (s_idx % GSUP)
            hT, tT = transposed.pop(s_idx)
            hA8 = hA8s.pop(s_idx)
            h16 = h8g[:, voff:voff + ST, :]
            t16 = t8g[:, voff:voff + ST, :]
            t16lo = t8log[:, voff:voff + ST, :]

            # One quad = the 4 tiles of this supertile. All [P,1]-sized gate
            # and copy work is batched across the quad to amortize per-inst
            # overhead; wp4 strides each tile's workspace by 256 f32 so no
            # matmul output crosses a PSUM bank boundary.
            # wp4[:, t, :] cols: 0:48 ps_hp, 48:96 ps_tp, 96 ws_s, 97 ws_c,
            # 98 gs, 99 gc, 100 S*den_s, 101 S*den_c, 102 hv, 103 tv,
            # 104:232 e^T (fp16 x256 via bitcast), 232:256 pad.
            tg0 = s_idx * ST
            wp4 = psw.tile([P, ST, 256], F32, tag="wp")
            ps2s = [pss.tile([P, 2, 2, P], F32, tag="ps", name=f"ps{s_idx}_{h}")
                    for h in range(2)]

            for t in range(ST):
                tg = tg0 + t
                # -- packed scores (8 batches x 16x16) + masks (one rank-10
                # matmul: rows 0-8 cross-batch block mask, row 9 key mask) --
                ps_pair = ps2s[t // 2][:, t % 2]
                ps_s = ps_pair[:, 0, :]
                ps_c = ps_pair[:, 1, :]
                for cc in range(KC2):
                    nc.tensor.matmul(ps_s, hA8[:, 2 * cc:2 * cc + 2, ts(t, P)],
                                     f8v(hT, cc)[:, :, ts(t, P)],
                                     start=(cc == 0), stop=False, perf_mode=DR)
                nc.tensor.matmul(ps_s, c10[:], vs_all[:, ts(tg, P)],
                                 start=False, stop=True)
                for cc in range(KC2):
                    nc.tensor.matmul(ps_c,
                                     hA8[:, KC + 2 * cc:KC + 2 * cc + 2, ts(t, P)],
                                     f8v(tT, cc)[:, :, ts(t, P)],
                                     start=(cc == 0), stop=False, perf_mode=DR)
                nc.tensor.matmul(ps_c, c10[:], vc_all[:, ts(tg, P)],
                                 start=False, stop=True)

                # -- gate dot inputs: hv = head@u, tv = tail@w2 (key-major,
                # 1-column DoubleRow matmuls, ~free on the PE); u and w2 are
                # fp8 hi+lo pairs (the gate is the precision-dominant path) --
                for k in range(4 * KC2):
                    cc, i, r = k // 4, (k // 2) % 2, k % 2
                    nc.tensor.matmul(wp4[:, t, 102:103],
                                     f8v(hT, cc)[:, i, ts(t, P)],
                                     acat8[:, cc, i, 1536 + r:1537 + r],
                                     start=(k == 0), stop=(k == 4 * KC2 - 1))
                for k in range(4 * KC2):
                    cc, i, r = k // 4, (k // 2) % 2, k % 2
                    nc.tensor.matmul(wp4[:, t, 103:104],
                                     f8v(tT, cc)[:, i, ts(t, P)],
                                     w2c8[:, cc, i, r:r + 1],
                                     start=(k == 0), stop=(k == 4 * KC2 - 1))

            # -- softmax numerators (free axis), one exp per 2 tiles;
            # masked lanes are ~-2e3 after the 1/S exp scale --
            e2s = []
            for h in range(2):
                e2 = pt.tile([P, 2, 2, P], F16, tag=f"e2_{h}", bufs=3)
                nc.scalar.activation(out=e2[:], in_=ps2s[h][:], func=Exp,
                                     bias=0.0, scale=1.0 / S)
                e2s.append(e2)

            # -- e^T on the PE so den/gs become 1-column matmuls --
            for t in range(ST):
                eT = wp4[:, t, 104:232].bitcast(F16).rearrange(
                    "p (c n) -> p c n", c=2)
                e_pair = e2s[t // 2][:, t % 2]
                nc.tensor.transpose(eT[:, 0, :], e_pair[:, 0, :], identh[:])
                nc.tensor.transpose(eT[:, 1, :], e_pair[:, 1, :], identh[:])

            # -- batched PSUM->SBUF staging for the whole quad --
            esT4 = pt.tile([P, ST, 2, P], F16, tag="esT", bufs=2)
            nc.vector.tensor_copy(
                esT4[:], wp4[:, :, 104:232].bitcast(F16).rearrange(
                    "p t (c n) -> p t c n", c=2))
            hvtv4 = pt.tile([P, ST, 2], F16, tag="hvtv", bufs=2)
            nc.vector.tensor_copy(hvtv4[:], wp4[:, :, 102:104])

            # cols: 98 gs_num, 99 gc_num, 100 S*den_s, 101 S*den_c
            for t in range(ST):
                nc.tensor.matmul(wp4[:, t, 98:99], esT4[:, t, 0, :],
                                 hvtv4[:, t, 0:1], start=True, stop=True)
                nc.tensor.matmul(wp4[:, t, 99:100], esT4[:, t, 1, :],
                                 hvtv4[:, t, 1:2], start=True, stop=True)
                nc.tensor.matmul(wp4[:, t, 100:101], esT4[:, t, 0, :],
                                 onesc[:], start=True, stop=True)
                nc.tensor.matmul(wp4[:, t, 101:102], esT4[:, t, 1, :],
                                 onesc[:], start=True, stop=True)

            # -- batched gate math (gate-as-exp; sign flips ride the host
            # constants onesc=-S, mw=-S*mw, c0=-C0 so the ACT engine only
            # ever needs the Exp table) --
            rden4 = pt.tile([P, ST, 2], F32, tag="rden", bufs=2)
            nc.vector.reciprocal(out=rden4[:], in_=wp4[:, :, 100:102])
            m4 = pt.tile([P, ST, 2], F32, tag="m4", bufs=2)
            nc.vector.tensor_mul(out=m4[:], in0=wp4[:, :, 98:100], in1=rden4[:])
            garg4 = pt.tile([P, ST], F32, tag="garg", bufs=2)
            nc.vector.tensor_add(out=garg4[:], in0=m4[:, :, 0], in1=m4[:, :, 1])
            eg4 = pt.tile([P, ST], F32, tag="eg", bufs=2)
            nc.scalar.activation(out=eg4[:], in_=garg4[:], func=Exp,
                                 bias=c0[:, 0:1], scale=1.0)
            gp4 = pt.tile([P, ST], F32, tag="gp", bufs=2)
            nc.vector.tensor_scalar_add(out=gp4[:], in0=eg4[:], scalar1=1.0)
            gate4 = pt.tile([P, ST], F32, tag="gate", bufs=2)
            nc.vector.reciprocal(out=gate4[:], in_=gp4[:])

            # -- pooling coefficient vectors (fold S*mw and 1/(S den)) --
            mw4 = mw_all[:, tg0:tg0 + ST]
            mwg4 = pt.tile([P, ST], F16, tag="mwg", bufs=2)   # S*mw*gate
            nc.vector.tensor_mul(out=mwg4[:], in0=mw4, in1=gate4[:])
            a_s4 = pt.tile([P, ST], F16, tag="a_s", bufs=2)
            nc.vector.tensor_mul(out=a_s4[:], in0=mwg4[:], in1=rden4[:, :, 0])
            mwc4 = pt.tile([P, ST], F16, tag="mwc", bufs=2)   # S*mw*(1-gate)
            nc.vector.tensor_sub(out=mwc4[:], in0=mw4, in1=mwg4[:])
            a_c4 = pt.tile([P, ST], F16, tag="a_c", bufs=2)
            nc.vector.tensor_mul(out=a_c4[:], in0=mwc4[:], in1=rden4[:, :, 1])

            # -- ws = e^T @ a : per-key pooled weights (block-diag safe) --
            for t in range(ST):
                e_pair = e2s[t // 2][:, t % 2]
                nc.tensor.matmul(wp4[:, t, 96:97], e_pair[:, 0, :],
                                 a_s4[:, t:t + 1], start=True, stop=True)
                nc.tensor.matmul(wp4[:, t, 97:98], e_pair[:, 1, :],
                                 a_c4[:, t:t + 1], start=True, stop=True)

            # -- block-diagonal weight columns via onehot, whole quad --
            diag_s4 = pt.tile([P, ST, 8], F16, tag="diag_s", bufs=2)
            diag_c4 = pt.tile([P, ST, 8], F16, tag="diag_c", bufs=2)
            nc.vector.tensor_tensor(out=diag_s4[:],
                                    in0=wp4[:, :, 96:97].to_broadcast([P, ST, 8]),
                                    in1=onehot4[:], op=mult)
            nc.vector.tensor_tensor(out=diag_c4[:],
                                    in0=wp4[:, :, 97:98].to_broadcast([P, ST, 8]),
                                    in1=onehot4[:], op=mult)

            # -- pools: feature-major pooled vectors, 8 batches per tile --
            for t in range(ST):
                ps_hp = wp4[:, t, 0:48].rearrange("p (c e) -> p c e", e=8)
                ps_tp = wp4[:, t, 48:96].rearrange("p (c e) -> p c e", e=8)
                for c in range(KC):
                    nc.tensor.matmul(ps_hp[:, c, :], h16[:, t, ts(c, P)],
                                     diag_s4[:, t, :], start=True, stop=True)
                    nc.tensor.matmul(ps_tp[:, c, :], t16[:, t, ts(c, P)],
                                     diag_c4[:, t, :], start=True, stop=False)
                    nc.tensor.matmul(ps_tp[:, c, :], t16lo[:, t, ts(c, P)],
                                     diag_c4[:, t, :], start=False, stop=True)
            nc.vector.tensor_copy(
                hp_all[:, :, tg0 * 8:(tg0 + ST) * 8].rearrange(
                    "p c (t e) -> p t c e", e=8),
                wp4[:, :, 0:48].rearrange("p t (c e) -> p t c e", e=8))
            nc.vector.tensor_copy(
                tp_all[:, :, tg0 * 8:(tg0 + ST) * 8].rearrange(
                    "p c (t e) -> p t c e", e=8),
                wp4[:, :, 48:96].rearrange("p t (c e) -> p t c e", e=8))

        for s_idx in range(NSUP):
            if s_idx == NSUP // 2:
                emit_final(0)
            if s_idx % GSUP == 0 and s_idx // GSUP + 1 < NSUP // GSUP:
                emit_loads16(s_idx // GSUP + 1)
                emit_loads8(s_idx // GSUP + 1)
            if s_idx + 1 < NSUP:
                emit_transpose(s_idx + 1)
            emit_gemm(s_idx)
            emit_tiles(s_idx)

        emit_final(1)


_NC_CACHE = None


def _get_nc():
    global _NC_CACHE
    if _NC_CACHE is None:
        _NC_CACHE = _build_nc()
    return _NC_CACHE


def _host_prep(Wsq, Wsk, Wsv, Wcq, Wck, Wg, bg, bsv,
               head_mask, tail_mask):
    """Fold weights; build per-core constant tensors (shared across cores
    except the mask-derived ones)."""
    f64 = np.float64
    scale = 1.0 / np.sqrt(f64(H))
    A_s = (Wsq.astype(f64).T @ Wsk.astype(f64)) * scale
    A_c = (Wcq.astype(f64).T @ Wck.astype(f64)) * scale
    A = np.concatenate([A_s, A_c], axis=1)                         # [768, 1536]
    # per 256-block of output features: even columns then odd columns, so
    # the big GEMM's PSUM chunk pairs (2c, 2c+1) hold features 256c+2p+i
    colperm = np.concatenate([
        np.concatenate([np.arange(256 * b, 256 * b + 256, 2),
                        np.arange(256 * b + 1, 256 * b + 256, 2)])
        for b in range(2 * H // 256)])
    Wg1 = Wg[0, :H].astype(f64)
    w2 = Wg[0, H:].astype(f64)
    u = Wsv.astype(f64).T @ Wg1
    uS = S * u
    u_hi = (uS.astype(np.float32)).astype(NP8)
    u_lo = uS - u_hi.astype(f64)                  # quantized again by the cast
    w2S = S * w2
    w2_hi = (w2S.astype(np.float32)).astype(NP8)
    w2_lo = w2S - w2_hi.astype(f64)
    acat = np.concatenate([A[:, colperm], u_hi.astype(f64)[:, None] / S,
                           u_lo[:, None] / S,
                           np.zeros((H, APAD - ACOLS - 1))], axis=1)  # [768, 1552]
    # rows (input features) interleaved: acat8[c, p, i] = S*acat[256c+2p+i]
    acat8 = (S * acat).reshape(KC2, P, 2, APAD).astype(NP8)
    w2_8 = np.stack([w2_hi.astype(f64), w2_lo], axis=-1)
    w2_8 = (w2_8).reshape(KC2, P, 2, 2).astype(NP8)
    wsvT_t = Wsv.astype(f64).T.reshape(KC, P, H).astype(np.float16)

    g = np.arange(P) // M                                          # group id per row
    # rank-10 combined mask operand: rows 0-8 = cross-batch block mask
    # (NEG everywhere, un-NEG within own 16-row block), row 9 = key mask
    c10 = np.zeros((10, P), np.float16)
    c9r = np.zeros((9, P), np.float16)
    c10[0] = 1.0
    c9r[0] = NEG
    for k in range(8):
        c10[1 + k] = (g == k).astype(np.float16)
        c9r[1 + k] = -NEG * (g == k).astype(np.float16)
    c10[9] = 1.0
    onesc = np.full((P, 1), -S, np.float16)   # negative: see gate-as-exp note
    onehot = np.zeros((P, 8), np.float16)
    onehot[np.arange(P), g] = 1.0
    onehot = np.tile(onehot, (1, 4))          # one copy per quad tile

    C0 = float(bg[0] + f64(bsv) @ Wg1)
    c0 = np.full((P, 1), -C0, np.float32)     # negated: gate-as-exp
    identh = np.eye(P, dtype=np.float16)

    # per-core mask-derived tensors: [10, TILES*P] moving operands whose
    # rows 0-8 repeat c9r every tile and row 9 carries the key mask
    hm = head_mask.reshape(NCORES, BC, M)
    tm = tail_mask.reshape(NCORES, BC, N)
    c9r_t = np.tile(c9r[:, None, :], (1, TILES, 1)).reshape(9, TILES * P)
    vs, vc, mw = [], [], []
    for i in range(NCORES):
        vsi = ((1 - hm[i]).astype(np.float16) * np.float16(NEG)
               ).reshape(1, TILES * P)
        vci = ((1 - tm[i]).astype(np.float16) * np.float16(NEG)
               ).reshape(1, TILES * P)
        vs.append(np.concatenate([c9r_t, vsi], axis=0))
        vc.append(np.concatenate([c9r_t, vci], axis=0))
        e = np.exp(hm[i].astype(f64))
        mwi = (-S * e / e.sum(axis=1, keepdims=True)).astype(np.float32)
        mw.append(mwi.reshape(TILES, P).T.copy())                    # [P, TILES]
    shared = dict(acat=acat8, w2c=w2_8, wsvT=wsvT_t, c10=c10,
                  onesc=onesc, onehot=onehot,
                  identh=identh, c0=c0)
    return shared, vs, vc, mw


def _core_feeds(head_mentions, tail_mentions, shared, vs, vc, mw, i):
    hm = head_mentions.reshape(NCORES, ROWS, H)
    tm = tail_mentions.reshape(NCORES, ROWS, H)
    t = np.ascontiguousarray(tm[i])
    t8 = t.astype(NP8)                       # hi fp8 (host cast, exact pair)
    tlo = t - t8.astype(np.float32)          # residual, SWDGE-cast to fp8
    feeds = {"head": np.ascontiguousarray(hm[i]),
             "tail8": t8, "taillo": tlo,
             "vs": vs[i], "vc": vc[i], "mw": mw[i]}
    feeds.update(shared)
    return feeds


def _reference_numpy(head_mentions, tail_mentions, head_mask, tail_mask,
                     Wsq, bsq, Wsk, bsk, Wsv, bsv, Wcq, bcq, Wck, bck, Wg, bg):
    """Exact fallback (only used if projection biases are nonzero)."""
    f = np.float32
    scale = f(1.0) / np.sqrt(f(H))
    hm = head_mentions.astype(f)
    tm = tail_mentions.astype(f)
    sq = hm @ Wsq.T + bsq
    sk = hm @ Wsk.T + bsk
    sv = hm @ Wsv.T + bsv
    ss = np.einsum("bmh,bnh->bmn", sq, sk) * scale
    ss = np.where(head_mask[:, None, :] == 0, f(NEG), ss)
    ss = ss - ss.max(-1, keepdims=True)
    e = np.exp(ss)
    sw = e / e.sum(-1, keepdims=True)
    self_out = np.einsum("bmn,bnh->bmh", sw, sv)
    cq = hm @ Wcq.T + bcq
    ck = tm @ Wck.T + bck
    cs = np.einsum("bmh,bnh->bmn", cq, ck) * scale
    cs = np.where(tail_mask[:, None, :] == 0, f(NEG), cs)
    cs = cs - cs.max(-1, keepdims=True)
    ec = np.exp(cs)
    cw = ec / ec.sum(-1, keepdims=True)
    cross_out = np.einsum("bmn,bnh->bmh", cw, tm)
    gate_in = np.concatenate([self_out, cross_out], axis=-1)
    gate = 1.0 / (1.0 + np.exp(-(np.einsum("bmh,oh->bmo", gate_in, Wg) + bg)))
    fused = gate * self_out + (1 - gate) * cross_out
    mexp = np.exp(head_mask.astype(f))
    mw = (mexp / mexp.sum(1, keepdims=True))[:, :, None]
    return (fused * mw).sum(axis=1)


def kernel(head_mentions, tail_mentions, head_mask, tail_mask,
           Wsq, bsq, Wsk, bsk, Wsv, bsv, Wcq, bcq, Wck, bck, Wg, bg,
           _trace=False):
    head_mentions = np.asarray(head_mentions)
    tail_mentions = np.asarray(tail_mentions)
    head_mask = np.asarray(head_mask)
    tail_mask = np.asarray(tail_mask)
    args = dict(Wsq=np.asarray(Wsq), bsq=np.asarray(bsq), Wsk=np.asarray(Wsk),
                bsk=np.asarray(bsk), Wsv=np.asarray(Wsv), bsv=np.asarray(bsv),
                Wcq=np.asarray(Wcq), bcq=np.asarray(bcq), Wck=np.asarray(Wck),
                bck=np.asarray(bck), Wg=np.asarray(Wg), bg=np.asarray(bg))

    # The folded formulation absorbs bg/bsv exactly; nonzero Q/K-side biases
    # (never produced by this problem's setup) would change the softmax and
    # are handled by the exact numpy fallback.
    if any(np.any(args[k] != 0) for k in ("bsq", "bsk", "bcq", "bck")):
        return _reference_numpy(head_mentions, tail_mentions, head_mask,
                                tail_mask, **args).astype(np.float32)

    shared, vs, vc, mw = _host_prep(args["Wsq"], args["Wsk"], args["Wsv"],
                                    args["Wcq"], args["Wck"], args["Wg"],
                                    args["bg"], args["bsv"],
                                    head_mask, tail_mask)

    nc = _get_nc()
    in_maps = [_core_feeds(head_mentions, tail_mentions, shared, vs, vc, mw, i)
               for i in range(NCORES)]
    res = run_bass_kernel_spmd(nc, in_maps, core_ids=list(range(NCORES)),
                               trace=_trace)
    out = np.concatenate([res.results[i]["out"] for i in range(NCORES)], axis=0)
    if _trace:
        kernel._last_result = res
    return out.astype(np.float32)



# revision 25
# speedup vs baseline: 1.0207x; 1.0207x over previous
"""Trainium2 Bass kernel for nn_EntityMentionAggregation.

Reference computation (per batch b, M=N=16 mentions, H=768):
  self-attn over head mentions, cross-attn head->tail, sigmoid-gated fusion,
  mask-softmax pooling over mentions -> out [B, H].

Algebraic restructuring (exact, given the zero biases produced by
setup_inputs; nonzero projection biases fall back to numpy):
  s_scores = scale * head @ (Wsq^T Wsk) @ head^T          (A_s folded)
  c_scores = scale * head @ (Wcq^T Wck) @ tail^T          (A_c folded)
  out      = hpool @ Wsv^T + tpool
    hpool  = ws_s^T-weighted sum of head rows, ws_s = s_w^T (mw*gate/den_s)
    tpool  = ws_c^T-weighted sum of tail rows
  gate     = sigmoid(s_w@(head@u) + c_w@(tail@w2) + C0), u = Wsv^T Wg1
so the V projection runs on pooled vectors (16x fewer rows) and
self_out/cross_out are never materialized.

Precision split: the score path (big GEMM + packed per-tile attention
matmuls + gate dot-products) runs in fp8 e4m3 with DoubleRow perf mode
(2 k-tiles of 128 per matmul at 0.5 cyc/row). On the value path the TAIL
pools stay fp16 (tpool lands raw in the output; fp8 there costs 3.5e-2
rel err) while the HEAD pools reuse the fp8 copy (hpool's quantization
noise washes through the Wsv^T projection; 1.35e-2 total, under the
2e-2 gate), which drops the fp16 head load entirely. The fp8 operands are
produced by a second SWDGE cast-load (f32->fp8) and transposed to
feature-major via the SBUF xbar with PAIRS of fp8 values packed in one
uint16 element; the resulting [feat-pair partition, 2, row] layout is
exactly DoubleRow's expected [K,2,N] k-tile shape (logical feature
f = 256c + 2p + i).  The folded A matrix is stored column-permuted
(per 256-block: even columns then odd columns) so the big GEMM's PSUM
partitions line up with the same pairing when its output chunks are used
as score-matmul weights.

Gate path: e = exp(scores/S) is transposed on the PE (identity matmul) so
den = e^T @ (-S*ones) and gs = e^T @ (head@u) become 1-column matmuls,
removing the partition-broadcast DRAM round-trip of hv entirely. The
sigmoid is evaluated as 1/(1+exp(-garg)) so the ACT engine only ever
needs the Exp table (Sigmoid lives in a different act-table set; each
switch would cost a 1.3us table reload); the sign flips ride host
constants (onesc=-S, mw=-S*mw, c0=-C0) and cancel in a_s/a_c. u and w2
are stored as fp8 hi+lo residual pairs - the gate dot-products were the
precision-dominant path (rel err 1.26e-2 -> 6.7e-3 on HW).
Scores carry a factor S=32 (folded into A) so the fp8 tensors sit inside
e4m3's dynamic range (max 240); exp applies scale 1/S.

Layout: batch is sharded 8 ways (512 batches/core); rows are processed in
tiles of 128 = 8 batches x 16 mentions. Cross-batch blocks are masked to
-inf via a rank-9 constant matmul and the per-tile key-mask via a rank-1
matmul, so softmax zeroes them exactly and the packed attention matrix is
block-diagonal -- which makes the pooling contractions plain matmuls
against block-diagonal weight columns built with a onehot multiply.
"""

import numpy as np
import ml_dtypes
import bass_rust
import concourse.bass as bass
import concourse.mybir as mybir
import concourse.tile as tile
from concourse.bass import ts
from concourse.bass_utils import run_bass_kernel_spmd

H = 768
B, M, N = 4096, 16, 16
NEG = -65504.0
P = 128
NCORES = 8
BC = B // NCORES          # batches per core = 512
ROWS = BC * M             # rows per core = 8192
TILES = ROWS // P         # 64 tiles (8 batches each)
ST = 4                    # tiles per supertile (GEMM moving N = 512)
NSUP = TILES // ST        # 16 supertiles
SN = ST * P               # 512 rows per supertile
GN = 512                  # GEMM moving width per PSUM pass (one bank)
KC = H // P               # 6 contraction chunks (128 each)
KC2 = H // (2 * P)        # 3 DoubleRow chunk pairs (256 each)
FO = 2 * H // P           # 12 score-feature chunks (A_s | A_c)
ACOLS = 2 * H + 1         # 1537: A_s | A_c | u
APAD = 1552               # ACOLS padded so the DoubleRow pair step is 16B-aligned
RPB = ROWS // M           # 512 pooled rows (batches) per core
S = 32.0                  # fp8 dynamic-range scale folded into A/u/w2/mw

F8 = mybir.dt.float8e4
F16 = mybir.dt.float16
F32 = mybir.dt.float32
U16 = mybir.dt.uint16
DR = mybir.MatmulPerfMode.DoubleRow
NP8 = ml_dtypes.float8_e4m3


def _split_sync_waits(nc):
    """Walrus caps sync waits per instruction (1 is the only universally
    accepted count in this toolchain). Hoist excess waits onto preceding
    single-wait EventSemaphore carriers on the same engine."""
    for f in nc.m.functions:
        for bb in f.blocks:
            il = bb.instructions
            new_il = []
            changed = False
            for inst in il:
                si = inst.sync_info
                if si is not None and len(si.on_wait) > 1:
                    waits = list(si.on_wait)
                    k = 0
                    while len(waits) > 1:
                        w, waits = waits[0], waits[1:]
                        d = bass_rust.InstEventSemaphore(
                            name=f"{inst.name}-wsplit{k}", ins=[], outs=[])
                        d.engine = inst.engine
                        d.sync_info = bass_rust.SyncInfo(on_wait=[w], on_update=[])
                        new_il.append(d)
                        k += 1
                        changed = True
                    inst.sync_info = bass_rust.SyncInfo(
                        on_wait=waits, on_update=list(si.on_update))
                new_il.append(inst)
            if changed:
                bb.instructions = new_il


def _build_nc(split=True):
    nc = bass.Bass(target_bir_lowering=False)

    head_d = nc.dram_tensor("head", [ROWS, H], F32, kind="ExternalInput")
    tail8_d = nc.dram_tensor("tail8", [ROWS, H], F8, kind="ExternalInput")
    taillo_d = nc.dram_tensor("taillo", [ROWS, H], F32, kind="ExternalInput")
    acat_d = nc.dram_tensor("acat", [KC2, P, 2, APAD], F8, kind="ExternalInput")
    w2_d = nc.dram_tensor("w2c", [KC2, P, 2, 2], F8, kind="ExternalInput")
    wsvT_d = nc.dram_tensor("wsvT", [KC, P, H], F16, kind="ExternalInput")
    c10_d = nc.dram_tensor("c10", [10, P], F16, kind="ExternalInput")
    onesc_d = nc.dram_tensor("onesc", [P, 1], F16, kind="ExternalInput")
    onehot_d = nc.dram_tensor("onehot", [P, ST * 8], F16, kind="ExternalInput")
    vs_d = nc.dram_tensor("vs", [10, TILES * P], F16, kind="ExternalInput")
    vc_d = nc.dram_tensor("vc", [10, TILES * P], F16, kind="ExternalInput")
    mw_d = nc.dram_tensor("mw", [P, TILES], F32, kind="ExternalInput")
    identh_d = nc.dram_tensor("identh", [P, P], F16, kind="ExternalInput")
    c0_d = nc.dram_tensor("c0", [P, 1], F32, kind="ExternalInput")
    out_d = nc.dram_tensor("out", [BC, H], F32, kind="ExternalOutput")

    with tile.TileContext(nc) as tc:
        _emit(nc, tc, head_d, tail8_d, taillo_d, acat_d, w2_d, wsvT_d, c10_d,
              onesc_d, onehot_d, vs_d, vc_d, mw_d,
              identh_d, c0_d, out_d)
    if split:
        _split_sync_waits(nc)
    return nc


def _emit(nc, tc, head_d, tail8_d, taillo_d, acat_d, w2_d, wsvT_d, c10_d,
          onesc_d, onehot_d, vs_d, vc_d, mw_d,
          identh_d, c0_d, out_d):
    from contextlib import ExitStack
    Exp = mybir.ActivationFunctionType.Exp
    Sig = mybir.ActivationFunctionType.Sigmoid
    Ident = mybir.ActivationFunctionType.Identity
    mult = mybir.AluOpType.mult
    ctx = ExitStack()
    with ctx:
        const = ctx.enter_context(tc.tile_pool(name="const", bufs=1))
        sup = ctx.enter_context(tc.tile_pool(name="sup", bufs=2))
        pt = ctx.enter_context(tc.tile_pool(name="pt", bufs=8))
        acc = ctx.enter_context(tc.tile_pool(name="acc", bufs=1))
        psg = ctx.enter_context(tc.tile_pool(name="psg", bufs=2, space="PSUM"))
        pss = ctx.enter_context(tc.tile_pool(name="pss", bufs=2, space="PSUM"))
        psw = ctx.enter_context(tc.tile_pool(name="psw", bufs=2, space="PSUM"))

        # ---- constants ----
        acat8 = const.tile([P, KC2, 2, APAD], F8)
        nc.sync.dma_start(out=acat8[:], in_=acat_d.rearrange("c p i m -> p c i m"))
        w2c8 = const.tile([P, KC2, 2, 2], F8)
        nc.sync.dma_start(out=w2c8[:], in_=w2_d.rearrange("c p i m -> p c i m"))
        c10 = const.tile([10, P], F16)
        nc.sync.dma_start(out=c10[:], in_=c10_d[:, :])
        onesc = const.tile([P, 1], F16)
        nc.sync.dma_start(out=onesc[:], in_=onesc_d[:, :])
        onehot4 = const.tile([P, ST, 8], F16)
        nc.sync.dma_start(out=onehot4[:], in_=onehot_d.rearrange(
            "p (t e) -> p t e", e=8))
        vs_all = const.tile([10, TILES * P], F16)
        nc.sync.dma_start(out=vs_all[:], in_=vs_d[:, :])
        vc_all = const.tile([10, TILES * P], F16)
        nc.sync.dma_start(out=vc_all[:], in_=vc_d[:, :])

        # ---- per-core accumulators ----
        hp_all = acc.tile([P, KC, RPB], F16)   # pooled head, feature-major
        tp_all = acc.tile([P, KC, RPB], F16)   # pooled tail, feature-major

        # loads are batched per PAIR of supertiles: the SWDGE descriptor-gen
        # time on the Pool engine is ~1-2.4us per instruction regardless of
        # size, and 4 cast-loads/supertile made Pool the DMA-issue serializer
        SG = 2 * ST
        GSUP = SG // ST
        head_r = head_d.rearrange("(g t p) h -> g p t h", t=SG, p=P)
        tail8_r = tail8_d.rearrange("(g t p) h -> g p t h", t=SG, p=P)
        taillo_r = taillo_d.rearrange("(g t p) h -> g p t h", t=SG, p=P)

        loaded16 = {}
        loaded8 = {}

        def emit_loads16(g):
            # fp8 LO residual of tail (host-split): tail = t8 + t8lo to
            # ~2^-8 relative, which the TAIL value pools need (tpool lands
            # raw in the output; single fp8 there costs 3.5e-2 rel err).
            # The HEAD value pools read the fp8 h8 copy alone -- hpool's
            # quantization noise washes through the Wsv^T projection.
            t8lo = sup.tile([P, SG, H], F8, tag="t8lo", name=f"t8lo_{g}",
                            bufs=3)
            nc.gpsimd.dma_start(out=t8lo[:], in_=taillo_r[g])
            loaded16[g] = t8lo

        def emit_loads8(g):
            # fp8 copies feed the score-side GEMMs; h8 cast from the f32
            # rows by SWDGE, t8 pre-cast on host (so the lo residual pairs
            # exactly)
            h8 = sup.tile([P, SG, H], F8, tag="h8", name=f"h8_{g}", bufs=3)
            t8 = sup.tile([P, SG, H], F8, tag="t8", name=f"t8_{g}", bufs=3)
            nc.gpsimd.dma_start(out=h8[:], in_=head_r[g])
            nc.gpsimd.dma_start(out=t8[:], in_=tail8_r[g])
            loaded8[g] = (h8, t8)

        transposed = {}

        def emit_transpose(s):
            # xbar transpose to feature-major with fp8 PAIRS packed in uint16
            h8, t8 = loaded8[s // GSUP]
            off = ST * (s % GSUP)
            hT = sup.tile([P, KC2, SN], U16, tag="hT", name=f"hT{s}", bufs=3)
            tT = sup.tile([P, KC2, SN], U16, tag="tT", name=f"tT{s}", bufs=3)
            for t in range(ST):
                nc.sync.dma_start_transpose(hT[:, :, ts(t, P)],
                                            h8[:, off + t, :].bitcast(U16))
                nc.sync.dma_start_transpose(tT[:, :, ts(t, P)],
                                            t8[:, off + t, :].bitcast(U16))
            transposed[s] = (hT, tT)

        def f8v(tT_, cc):
            # DoubleRow moving view of a pair-packed chunk: [K=128, 2, n]
            return tT_[:, cc, :].bitcast(F8).rearrange(
                "p (n two) -> p two n", two=2)

        out_sb = acc.tile([P, BC // P, H], F32)
        out_r = out_d.rearrange("(r p) h -> p r h", p=P)

        def emit_final(half):
            # out[batch, :] = hpool @ Wsv^T + tpool, computed ROW-major:
            # hp_all chunks are the (free) stationary with batches on the
            # free axis, wsvT moving; tpool is transpose-accumulated into
            # the same PSUM group via an identity moving operand. One batched
            # ACT copy per PSUM bank evacuates, then DMA-store.
            for r in range(half * (BC // P // 2), (half + 1) * (BC // P // 2)):
                pA = psg.tile([P, GN], F32, tag="pg", name=f"pfA{r}")
                pB = psg.tile([P, GN], F32, tag="pg", name=f"pfB{r}")
                for j in range(KC):
                    tgt = pA[:, ts(j, P)] if j < 4 else pB[:, ts(j - 4, P)]
                    for c in range(KC):
                        nc.tensor.matmul(tgt, hp_all[:, c, ts(r, P)],
                                         wsvT[:, c, ts(j, P)],
                                         start=(c == 0), stop=False)
                    nc.tensor.matmul(tgt, tp_all[:, j, ts(r, P)], identh[:],
                                     start=False, stop=True)
                nc.scalar.copy(out_sb[:, r, 0:GN], pA[:])
                nc.scalar.copy(out_sb[:, r, GN:H], pB[:, :H - GN])
                nc.sync.dma_start(out=out_r[:, r, :], in_=out_sb[:, r, :])

        emit_loads16(0)
        emit_loads8(0)
        emit_transpose(0)
        wsvT = const.tile([P, KC, H], F16)
        nc.sync.dma_start(out=wsvT[:], in_=wsvT_d.rearrange("c p m -> p c m"))
        mw_all = const.tile([P, TILES], F32)
        nc.sync.dma_start(out=mw_all[:], in_=mw_d[:, :])
        identh = const.tile([P, P], F16)
        nc.sync.dma_start(out=identh[:], in_=identh_d[:, :])
        c0 = const.tile([P, 1], F32)
        nc.sync.dma_start(out=c0[:], in_=c0_d[:, :])
        hA8s = {}

        def emit_gemm(s):
            # -- big GEMM: hA = head @ [A_s | A_c], feature-major, fp8 DR --
            hT, tT = transposed[s]
            hA8 = sup.tile([P, FO, SN], F8, tag="hA8", name=f"hA8_{s}")
            for j in range(FO):
                for hh in range(SN // GN):
                    pg = psg.tile([P, GN], F32, tag="pg")
                    for cc in range(KC2):
                        nc.tensor.matmul(pg[:], acat8[:, cc, :, ts(j, P)],
                                         f8v(hT, cc)[:, :, ts(hh, GN)],
                                         start=(cc == 0),
                                         stop=(cc == KC2 - 1), perf_mode=DR)
                    if j < 7:
                        nc.scalar.copy(hA8[:, j, ts(hh, GN)], pg[:])
                    else:
                        nc.vector.tensor_copy(hA8[:, j, ts(hh, GN)], pg[:])
            hA8s[s] = hA8

        def emit_tiles(s_idx):
            t8log = loaded16[s_idx // GSUP]
            h8g, t8g = loaded8[s_idx // GSUP]
            voff = ST * (s_idx % GSUP)
            hT, tT = transposed.pop(s_idx)
            hA8 = hA8s.pop(s_idx)
            h16 = h8g[:, voff:voff + ST, :]
            t16 = t8g[:, voff:voff + ST, :]
            t16lo = t8log[:, voff:voff + ST, :]

            # One quad = the 4 tiles of this supertile. All [P,1]-sized gate
            # and copy work is batched across the quad to amortize per-inst
            # overhead; wp4 strides each tile's workspace by 256 f32 so no
            # matmul output crosses a PSUM bank boundary.
            # wp4[:, t, :] cols: 0:48 ps_hp, 48:96 ps_tp, 96 ws_s, 97 ws_c,
            # 98 gs, 99 gc, 100 S*den_s, 101 S*den_c, 102 hv, 103 tv,
            # 104:232 e^T (fp16 x256 via bitcast), 232:256 pad.
            tg0 = s_idx * ST
            wp4 = psw.tile([P, ST, 256], F32, tag="wp")
            ps2s = [pss.tile([P, 2, 2, P], F32, tag="ps", name=f"ps{s_idx}_{h}")
                    for h in range(2)]

            for t in range(ST):
                tg = tg0 + t
                # -- packed scores (8 batches x 16x16) + masks (one rank-10
                # matmul: rows 0-8 cross-batch block mask, row 9 key mask) --
                ps_pair = ps2s[t // 2][:, t % 2]
                ps_s = ps_pair[:, 0, :]
                ps_c = ps_pair[:, 1, :]
                for cc in range(KC2):
                    nc.tensor.matmul(ps_s, hA8[:, 2 * cc:2 * cc + 2, ts(t, P)],
                                     f8v(hT, cc)[:, :, ts(t, P)],
                                     start=(cc == 0), stop=False, perf_mode=DR)
                nc.tensor.matmul(ps_s, c10[:], vs_all[:, ts(tg, P)],
                                 start=False, stop=True)
                for cc in range(KC2):
                    nc.tensor.matmul(ps_c,
                                     hA8[:, KC + 2 * cc:KC + 2 * cc + 2, ts(t, P)],
                                     f8v(tT, cc)[:, :, ts(t, P)],
                                     start=(cc == 0), stop=False, perf_mode=DR)
                nc.tensor.matmul(ps_c, c10[:], vc_all[:, ts(tg, P)],
                                 start=False, stop=True)

                # -- gate dot inputs: hv = head@u, tv = tail@w2 (key-major,
                # 1-column DoubleRow matmuls, ~free on the PE); u and w2 are
                # fp8 hi+lo pairs (the gate is the precision-dominant path) --
                for k in range(4 * KC2):
                    cc, i, r = k // 4, (k // 2) % 2, k % 2
                    nc.tensor.matmul(wp4[:, t, 102:103],
                                     f8v(hT, cc)[:, i, ts(t, P)],
                                     acat8[:, cc, i, 1536 + r:1537 + r],
                                     start=(k == 0), stop=(k == 4 * KC2 - 1))
                for k in range(4 * KC2):
                    cc, i, r = k // 4, (k // 2) % 2, k % 2
                    nc.tensor.matmul(wp4[:, t, 103:104],
                                     f8v(tT, cc)[:, i, ts(t, P)],
                                     w2c8[:, cc, i, r:r + 1],
                                     start=(k == 0), stop=(k == 4 * KC2 - 1))

            # -- softmax numerators (free axis), one exp per 2 tiles;
            # masked lanes are ~-2e3 after the 1/S exp scale --
            e2s = []
            for h in range(2):
                e2 = pt.tile([P, 2, 2, P], F16, tag=f"e2_{h}", bufs=3)
                nc.scalar.activation(out=e2[:], in_=ps2s[h][:], func=Exp,
                                     bias=0.0, scale=1.0 / S)
                e2s.append(e2)

            # -- e^T on the PE so den/gs become 1-column matmuls --
            for t in range(ST):
                eT = wp4[:, t, 104:232].bitcast(F16).rearrange(
                    "p (c n) -> p c n", c=2)
                e_pair = e2s[t // 2][:, t % 2]
                nc.tensor.transpose(eT[:, 0, :], e_pair[:, 0, :], identh[:])
                nc.tensor.transpose(eT[:, 1, :], e_pair[:, 1, :], identh[:])

            # -- batched PSUM->SBUF staging for the whole quad --
            esT4 = pt.tile([P, ST, 2, P], F16, tag="esT", bufs=2)
            nc.vector.tensor_copy(
                esT4[:], wp4[:, :, 104:232].bitcast(F16).rearrange(
                    "p t (c n) -> p t c n", c=2))
            hvtv4 = pt.tile([P, ST, 2], F16, tag="hvtv", bufs=2)
            nc.vector.tensor_copy(hvtv4[:], wp4[:, :, 102:104])

            # cols: 98 gs_num, 99 gc_num, 100 S*den_s, 101 S*den_c
            for t in range(ST):
                nc.tensor.matmul(wp4[:, t, 98:99], esT4[:, t, 0, :],
                                 hvtv4[:, t, 0:1], start=True, stop=True)
                nc.tensor.matmul(wp4[:, t, 99:100], esT4[:, t, 1, :],
                                 hvtv4[:, t, 1:2], start=True, stop=True)
                nc.tensor.matmul(wp4[:, t, 100:101], esT4[:, t, 0, :],
                                 onesc[:], start=True, stop=True)
                nc.tensor.matmul(wp4[:, t, 101:102], esT4[:, t, 1, :],
                                 onesc[:], start=True, stop=True)

            # -- batched gate math (gate-as-exp; sign flips ride the host
            # constants onesc=-S, mw=-S*mw, c0=-C0 so the ACT engine only
            # ever needs the Exp table) --
            rden4 = pt.tile([P, ST, 2], F32, tag="rden", bufs=2)
            nc.vector.reciprocal(out=rden4[:], in_=wp4[:, :, 100:102])
            m4 = pt.tile([P, ST, 2], F32, tag="m4", bufs=2)
            nc.vector.tensor_mul(out=m4[:], in0=wp4[:, :, 98:100], in1=rden4[:])
            garg4 = pt.tile([P, ST], F32, tag="garg", bufs=2)
            nc.vector.tensor_add(out=garg4[:], in0=m4[:, :, 0], in1=m4[:, :, 1])
            eg4 = pt.tile([P, ST], F32, tag="eg", bufs=2)
            nc.scalar.activation(out=eg4[:], in_=garg4[:], func=Exp,
                                 bias=c0[:, 0:1], scale=1.0)
            gp4 = pt.tile([P, ST], F32, tag="gp", bufs=2)
            nc.vector.tensor_scalar_add(out=gp4[:], in0=eg4[:], scalar1=1.0)
            gate4 = pt.tile([P, ST], F32, tag="gate", bufs=2)
            nc.vector.reciprocal(out=gate4[:], in_=gp4[:])

            # -- pooling coefficient vectors (fold S*mw and 1/(S den)) --
            mw4 = mw_all[:, tg0:tg0 + ST]
            mwg4 = pt.tile([P, ST], F16, tag="mwg", bufs=2)   # S*mw*gate
            nc.vector.tensor_mul(out=mwg4[:], in0=mw4, in1=gate4[:])
            a_s4 = pt.tile([P, ST], F16, tag="a_s", bufs=2)
            nc.vector.tensor_mul(out=a_s4[:], in0=mwg4[:], in1=rden4[:, :, 0])
            mwc4 = pt.tile([P, ST], F16, tag="mwc", bufs=2)   # S*mw*(1-gate)
            nc.vector.tensor_sub(out=mwc4[:], in0=mw4, in1=mwg4[:])
            a_c4 = pt.tile([P, ST], F16, tag="a_c", bufs=2)
            nc.vector.tensor_mul(out=a_c4[:], in0=mwc4[:], in1=rden4[:, :, 1])

            # -- ws = e^T @ a : per-key pooled weights (block-diag safe) --
            for t in range(ST):
                e_pair = e2s[t // 2][:, t % 2]
                nc.tensor.matmul(wp4[:, t, 96:97], e_pair[:, 0, :],
                                 a_s4[:, t:t + 1], start=True, stop=True)
                nc.tensor.matmul(wp4[:, t, 97:98], e_pair[:, 1, :],
                                 a_c4[:, t:t + 1], start=True, stop=True)

            # -- block-diagonal weight columns via onehot, whole quad --
            diag_s4 = pt.tile([P, ST, 8], F16, tag="diag_s", bufs=2)
            diag_c4 = pt.tile([P, ST, 8], F16, tag="diag_c", bufs=2)
            nc.vector.tensor_tensor(out=diag_s4[:],
                                    in0=wp4[:, :, 96:97].to_broadcast([P, ST, 8]),
                                    in1=onehot4[:], op=mult)
            nc.vector.tensor_tensor(out=diag_c4[:],
                                    in0=wp4[:, :, 97:98].to_broadcast([P, ST, 8]),
                                    in1=onehot4[:], op=mult)

            # -- pools: feature-major pooled vectors, 8 batches per tile --
            for t in range(ST):
                ps_hp = wp4[:, t, 0:48].rearrange("p (c e) -> p c e", e=8)
                ps_tp = wp4[:, t, 48:96].rearrange("p (c e) -> p c e", e=8)
                for c in range(KC):
                    nc.tensor.matmul(ps_hp[:, c, :], h16[:, t, ts(c, P)],
                                     diag_s4[:, t, :], start=True, stop=True)
                    nc.tensor.matmul(ps_tp[:, c, :], t16[:, t, ts(c, P)],
                                     diag_c4[:, t, :], start=True, stop=False)
                    nc.tensor.matmul(ps_tp[:, c, :], t16lo[:, t, ts(c, P)],
                                     diag_c4[:, t, :], start=False, stop=True)
            nc.vector.tensor_copy(
                hp_all[:, :, tg0 * 8:(tg0 + ST) * 8].rearrange(
                    "p c (t e) -> p t c e", e=8),
                wp4[:, :, 0:48].rearrange("p t (c e) -> p t c e", e=8))
            nc.vector.tensor_copy(
                tp_all[:, :, tg0 * 8:(tg0 + ST) * 8].rearrange(
                    "p c (t e) -> p t c e", e=8),
                wp4[:, :, 48:96].rearrange("p t (c e) -> p t c e", e=8))

        for s_idx in range(NSUP):
            if s_idx == NSUP // 2:
                emit_final(0)
            if s_idx % GSUP == 0 and s_idx // GSUP + 1 < NSUP // GSUP:
                emit_loads16(s_idx // GSUP + 1)
                emit_loads8(s_idx // GSUP + 1)
            if s_idx + 1 < NSUP:
                emit_transpose(s_idx + 1)
            emit_gemm(s_idx)
            emit_tiles(s_idx)

        emit_final(1)


_NC_CACHE = None


def _get_nc():
    global _NC_CACHE
    if _NC_CACHE is None:
        _NC_CACHE = _build_nc()
    return _NC_CACHE


def _host_prep(Wsq, Wsk, Wsv, Wcq, Wck, Wg, bg, bsv,
               head_mask, tail_mask):
    """Fold weights; build per-core constant tensors (shared across cores
    except the mask-derived ones)."""
    f64 = np.float64
    scale = 1.0 / np.sqrt(f64(H))
    A_s = (Wsq.astype(f64).T @ Wsk.astype(f64)) * scale
    A_c = (Wcq.astype(f64).T @ Wck.astype(f64)) * scale
    A = np.concatenate([A_s, A_c], axis=1)                         # [768, 1536]
    # per 256-block of output features: even columns then odd columns, so
    # the big GEMM's PSUM chunk pairs (2c, 2c+1) hold features 256c+2p+i
    colperm = np.concatenate([
        np.concatenate([np.arange(256 * b, 256 * b + 256, 2),
                        np.arange(256 * b + 1, 256 * b + 256, 2)])
        for b in range(2 * H // 256)])
    Wg1 = Wg[0, :H].astype(f64)
    w2 = Wg[0, H:].astype(f64)
    u = Wsv.astype(f64).T @ Wg1
    uS = S * u
    u_hi = (uS.astype(np.float32)).astype(NP8)
    u_lo = uS - u_hi.astype(f64)                  # quantized again by the cast
    w2S = S * w2
    w2_hi = (w2S.astype(np.float32)).astype(NP8)
    w2_lo = w2S - w2_hi.astype(f64)
    acat = np.concatenate([A[:, colperm], u_hi.astype(f64)[:, None] / S,
                           u_lo[:, None] / S,
                           np.zeros((H, APAD - ACOLS - 1))], axis=1)  # [768, 1552]
    # rows (input features) interleaved: acat8[c, p, i] = S*acat[256c+2p+i]
    acat8 = (S * acat).reshape(KC2, P, 2, APAD).astype(NP8)
    w2_8 = np.stack([w2_hi.astype(f64), w2_lo], axis=-1)
    w2_8 = (w2_8).reshape(KC2, P, 2, 2).astype(NP8)
    wsvT_t = Wsv.astype(f64).T.reshape(KC, P, H).astype(np.float16)

    g = np.arange(P) // M                                          # group id per row
    # rank-10 combined mask operand: rows 0-8 = cross-batch block mask
    # (NEG everywhere, un-NEG within own 16-row block), row 9 = key mask
    c10 = np.zeros((10, P), np.float16)
    c9r = np.zeros((9, P), np.float16)
    c10[0] = 1.0
    c9r[0] = NEG
    for k in range(8):
        c10[1 + k] = (g == k).astype(np.float16)
        c9r[1 + k] = -NEG * (g == k).astype(np.float16)
    c10[9] = 1.0
    onesc = np.full((P, 1), -S, np.float16)   # negative: see gate-as-exp note
    onehot = np.zeros((P, 8), np.float16)
    onehot[np.arange(P), g] = 1.0
    onehot = np.tile(onehot, (1, 4))          # one copy per quad tile

    C0 = float(bg[0] + f64(bsv) @ Wg1)
    c0 = np.full((P, 1), -C0, np.float32)     # negated: gate-as-exp
    identh = np.eye(P, dtype=np.float16)

    # per-core mask-derived tensors: [10, TILES*P] moving operands whose
    # rows 0-8 repeat c9r every tile and row 9 carries the key mask
    hm = head_mask.reshape(NCORES, BC, M)
    tm = tail_mask.reshape(NCORES, BC, N)
    c9r_t = np.tile(c9r[:, None, :], (1, TILES, 1)).reshape(9, TILES * P)
    vs, vc, mw = [], [], []
    for i in range(NCORES):
        vsi = ((1 - hm[i]).astype(np.float16) * np.float16(NEG)
               ).reshape(1, TILES * P)
        vci = ((1 - tm[i]).astype(np.float16) * np.float16(NEG)
               ).reshape(1, TILES * P)
        vs.append(np.concatenate([c9r_t, vsi], axis=0))
        vc.append(np.concatenate([c9r_t, vci], axis=0))
        e = np.exp(hm[i].astype(f64))
        mwi = (-S * e / e.sum(axis=1, keepdims=True)).astype(np.float32)
        mw.append(mwi.reshape(TILES, P).T.copy())                    # [P, TILES]
    shared = dict(acat=acat8, w2c=w2_8, wsvT=wsvT_t, c10=c10,
                  onesc=onesc, onehot=onehot,
                  identh=identh, c0=c0)
    return shared, vs, vc, mw


def _core_feeds(head_mentions, tail_mentions, shared, vs, vc, mw, i):
    hm = head_mentions.reshape(NCORES, ROWS, H)
    tm = tail_mentions.reshape(NCORES, ROWS, H)
    t = np.ascontiguousarray(tm[i])
    t8 = t.astype(NP8)                       # hi fp8 (host cast, exact pair)
    tlo = t - t8.astype(np.float32)          # residual, SWDGE-cast to fp8
    feeds = {"head": np.ascontiguousarray(hm[i]),
             "tail8": t8, "taillo": tlo,
             "vs": vs[i], "vc": vc[i], "mw": mw[i]}
    feeds.update(shared)
    return feeds


def _reference_numpy(head_mentions, tail_mentions, head_mask, tail_mask,
                     Wsq, bsq, Wsk, bsk, Wsv, bsv, Wcq, bcq, Wck, bck, Wg, bg):
    """Exact fallback (only used if projection biases are nonzero)."""
    f = np.float32
    scale = f(1.0) / np.sqrt(f(H))
    hm = head_mentions.astype(f)
    tm = tail_mentions.astype(f)
    sq = hm @ Wsq.T + bsq
    sk = hm @ Wsk.T + bsk
    sv = hm @ Wsv.T + bsv
    ss = np.einsum("bmh,bnh->bmn", sq, sk) * scale
    ss = np.where(head_mask[:, None, :] == 0, f(NEG), ss)
    ss = ss - ss.max(-1, keepdims=True)
    e = np.exp(ss)
    sw = e / e.sum(-1, keepdims=True)
    self_out = np.einsum("bmn,bnh->bmh", sw, sv)
    cq = hm @ Wcq.T + bcq
    ck = tm @ Wck.T + bck
    cs = np.einsum("bmh,bnh->bmn", cq, ck) * scale
    cs = np.where(tail_mask[:, None, :] == 0, f(NEG), cs)
    cs = cs - cs.max(-1, keepdims=True)
    ec = np.exp(cs)
    cw = ec / ec.sum(-1, keepdims=True)
    cross_out = np.einsum("bmn,bnh->bmh", cw, tm)
    gate_in = np.concatenate([self_out, cross_out], axis=-1)
    gate = 1.0 / (1.0 + np.exp(-(np.einsum("bmh,oh->bmo", gate_in, Wg) + bg)))
    fused = gate * self_out + (1 - gate) * cross_out
    mexp = np.exp(head_mask.astype(f))
    mw = (mexp / mexp.sum(1, keepdims=True))[:, :, None]
    return (fused * mw).sum(axis=1)


def kernel(head_mentions, tail_mentions, head_mask, tail_mask,
           Wsq, bsq, Wsk, bsk, Wsv, bsv, Wcq, bcq, Wck, bck, Wg, bg,
           _trace=False):
    head_mentions = np.asarray(head_mentions)
    tail_mentions = np.asarray(tail_mentions)
    head_mask = np.asarray(head_mask)
    tail_mask = np.asarray(tail_mask)
    args = dict(Wsq=np.asarray(Wsq), bsq=np.asarray(bsq), Wsk=np.asarray(Wsk),
                bsk=np.asarray(bsk), Wsv=np.asarray(Wsv), bsv=np.asarray(bsv),
                Wcq=np.asarray(Wcq), bcq=np.asarray(bcq), Wck=np.asarray(Wck),
                bck=np.asarray(bck), Wg=np.asarray(Wg), bg=np.asarray(bg))

    # The folded formulation absorbs bg/bsv exactly; nonzero Q/K-side biases
    # (never produced by this problem's setup) would change the softmax and
    # are handled by the exact numpy fallback.
    if any(np.any(args[k] != 0) for k in ("bsq", "bsk", "bcq", "bck")):
        return _reference_numpy(head_mentions, tail_mentions, head_mask,
                                tail_mask, **args).astype(np.float32)

    shared, vs, vc, mw = _host_prep(args["Wsq"], args["Wsk"], args["Wsv"],
                                    args["Wcq"], args["Wck"], args["Wg"],
                                    args["bg"], args["bsv"],
                                    head_mask, tail_mask)

    nc = _get_nc()
    in_maps = [_core_feeds(head_mentions, tail_mentions, shared, vs, vc, mw, i)
               for i in range(NCORES)]
    res = run_bass_kernel_spmd(nc, in_maps, core_ids=list(range(NCORES)),
                               trace=_trace)
    out = np.concatenate([res.results[i]["out"] for i in range(NCORES)], axis=0)
    if _trace:
        kernel._last_result = res
    return out.astype(np.float32)



# revision 31
# speedup vs baseline: 1.1083x; 1.0858x over previous
"""Trainium2 Bass kernel for nn_EntityMentionAggregation.

Reference computation (per batch b, M=N=16 mentions, H=768):
  self-attn over head mentions, cross-attn head->tail, sigmoid-gated fusion,
  mask-softmax pooling over mentions -> out [B, H].

Algebraic restructuring (exact, given the zero biases produced by
setup_inputs; nonzero projection biases fall back to numpy):
  s_scores = scale * head @ (Wsq^T Wsk) @ head^T          (A_s folded)
  c_scores = scale * head @ (Wcq^T Wck) @ tail^T          (A_c folded)
  out      = hpool @ Wsv^T + tpool
    hpool  = ws_s^T-weighted sum of head rows, ws_s = s_w^T (mw*gate/den_s)
    tpool  = ws_c^T-weighted sum of tail rows
  gate     = sigmoid(s_w@(head@u) + c_w@(tail@w2) + C0), u = Wsv^T Wg1
so the V projection runs on pooled vectors (16x fewer rows) and
self_out/cross_out are never materialized.

Precision split: the score path (big GEMM + packed per-tile attention
matmuls + gate dot-products) runs in fp8 e4m3 with DoubleRow perf mode
(2 k-tiles of 128 per matmul at 0.5 cyc/row). On the value path the TAIL
pools stay fp16 (tpool lands raw in the output; fp8 there costs 3.5e-2
rel err) while the HEAD pools reuse the fp8 copy (hpool's quantization
noise washes through the Wsv^T projection; 1.35e-2 total, under the
2e-2 gate), which drops the fp16 head load entirely. The fp8 operands are
produced by a second SWDGE cast-load (f32->fp8) and transposed to
feature-major via the SBUF xbar with PAIRS of fp8 values packed in one
uint16 element; the resulting [feat-pair partition, 2, row] layout is
exactly DoubleRow's expected [K,2,N] k-tile shape (logical feature
f = 256c + 2p + i).  The folded A matrix is stored column-permuted
(per 256-block: even columns then odd columns) so the big GEMM's PSUM
partitions line up with the same pairing when its output chunks are used
as score-matmul weights.

Gate path: e = exp(scores/S) is transposed on the PE (identity matmul) so
den = e^T @ (-S*ones) and gs = e^T @ (head@u) become 1-column matmuls,
removing the partition-broadcast DRAM round-trip of hv entirely. The
sigmoid is evaluated as 1/(1+exp(-garg)) so the ACT engine only ever
needs the Exp table (Sigmoid lives in a different act-table set; each
switch would cost a 1.3us table reload); the sign flips ride host
constants (onesc=-S, mw=-S*mw, c0=-C0) and cancel in a_s/a_c. u and w2
are stored as fp8 hi+lo residual pairs - the gate dot-products were the
precision-dominant path (rel err 1.26e-2 -> 6.7e-3 on HW).
Scores carry a factor S=32 (folded into A) so the fp8 tensors sit inside
e4m3's dynamic range (max 240); exp applies scale 1/S.

Layout: batch is sharded 8 ways (512 batches/core); rows are processed in
tiles of 128 = 8 batches x 16 mentions. Cross-batch blocks are masked to
-inf via a rank-9 constant matmul and the per-tile key-mask via a rank-1
matmul, so softmax zeroes them exactly and the packed attention matrix is
block-diagonal -- which makes the pooling contractions plain matmuls
against block-diagonal weight columns built with a onehot multiply.
"""

import numpy as np
import ml_dtypes
import bass_rust
import concourse.bass as bass
import concourse.mybir as mybir
import concourse.tile as tile
from concourse.bass import ts
from concourse.bass_utils import run_bass_kernel_spmd

H = 768
B, M, N = 4096, 16, 16
NEG = -65504.0
P = 128
NCORES = 8
BC = B // NCORES          # batches per core = 512
ROWS = BC * M             # rows per core = 8192
TILES = ROWS // P         # 64 tiles (8 batches each)
ST = 4                    # tiles per supertile (GEMM moving N = 512)
NSUP = TILES // ST        # 16 supertiles
SN = ST * P               # 512 rows per supertile
GN = 512                  # GEMM moving width per PSUM pass (one bank)
KC = H // P               # 6 contraction chunks (128 each)
KC2 = H // (2 * P)        # 3 DoubleRow chunk pairs (256 each)
FO = 2 * H // P           # 12 score-feature chunks (A_s | A_c)
ACOLS = 2 * H + 1         # 1537: A_s | A_c | u
APAD = 1552               # ACOLS padded so the DoubleRow pair step is 16B-aligned
RPB = ROWS // M           # 512 pooled rows (batches) per core
S = 32.0                  # fp8 dynamic-range scale folded into A/u/w2/mw

F8 = mybir.dt.float8e4
F16 = mybir.dt.float16
F32 = mybir.dt.float32
U16 = mybir.dt.uint16
DR = mybir.MatmulPerfMode.DoubleRow
NP8 = ml_dtypes.float8_e4m3


def _split_sync_waits(nc):
    """Walrus caps sync waits per instruction (1 is the only universally
    accepted count in this toolchain). Hoist excess waits onto preceding
    single-wait EventSemaphore carriers on the same engine."""
    for f in nc.m.functions:
        for bb in f.blocks:
            il = bb.instructions
            new_il = []
            changed = False
            for inst in il:
                si = inst.sync_info
                if si is not None and len(si.on_wait) > 1:
                    waits = list(si.on_wait)
                    k = 0
                    while len(waits) > 1:
                        w, waits = waits[0], waits[1:]
                        d = bass_rust.InstEventSemaphore(
                            name=f"{inst.name}-wsplit{k}", ins=[], outs=[])
                        d.engine = inst.engine
                        d.sync_info = bass_rust.SyncInfo(on_wait=[w], on_update=[])
                        new_il.append(d)
                        k += 1
                        changed = True
                    inst.sync_info = bass_rust.SyncInfo(
                        on_wait=waits, on_update=list(si.on_update))
                new_il.append(inst)
            if changed:
                bb.instructions = new_il


def _build_nc(split=True):
    nc = bass.Bass(target_bir_lowering=False)

    head_d = nc.dram_tensor("head", [ROWS, H], F32, kind="ExternalInput")
    tail8_d = nc.dram_tensor("tail8", [ROWS, H], F8, kind="ExternalInput")
    taillo_d = nc.dram_tensor("taillo", [ROWS, H], F32, kind="ExternalInput")
    acat_d = nc.dram_tensor("acat", [KC2, P, 2, APAD], F8, kind="ExternalInput")
    w2_d = nc.dram_tensor("w2c", [KC2, P, 2, 2], F8, kind="ExternalInput")
    wsvT_d = nc.dram_tensor("wsvT", [KC, P, H], F16, kind="ExternalInput")
    c10_d = nc.dram_tensor("c10", [10, P], F16, kind="ExternalInput")
    onesc_d = nc.dram_tensor("onesc", [P, 1], F16, kind="ExternalInput")
    onehot_d = nc.dram_tensor("onehot", [P, ST * 8], F16, kind="ExternalInput")
    vs_d = nc.dram_tensor("vs", [10, TILES * P], F16, kind="ExternalInput")
    vc_d = nc.dram_tensor("vc", [10, TILES * P], F16, kind="ExternalInput")
    mw_d = nc.dram_tensor("mw", [P, TILES], F32, kind="ExternalInput")
    identh_d = nc.dram_tensor("identh", [P, P], F16, kind="ExternalInput")
    c0_d = nc.dram_tensor("c0", [P, 1], F32, kind="ExternalInput")
    out_d = nc.dram_tensor("out", [BC, H], F32, kind="ExternalOutput")

    with tile.TileContext(nc) as tc:
        _emit(nc, tc, head_d, tail8_d, taillo_d, acat_d, w2_d, wsvT_d, c10_d,
              onesc_d, onehot_d, vs_d, vc_d, mw_d,
              identh_d, c0_d, out_d)
    if split:
        _split_sync_waits(nc)
    return nc


def _emit(nc, tc, head_d, tail8_d, taillo_d, acat_d, w2_d, wsvT_d, c10_d,
          onesc_d, onehot_d, vs_d, vc_d, mw_d,
          identh_d, c0_d, out_d):
    from contextlib import ExitStack
    Exp = mybir.ActivationFunctionType.Exp
    Sig = mybir.ActivationFunctionType.Sigmoid
    Ident = mybir.ActivationFunctionType.Identity
    mult = mybir.AluOpType.mult
    ctx = ExitStack()
    with ctx:
        const = ctx.enter_context(tc.tile_pool(name="const", bufs=1))
        sup = ctx.enter_context(tc.tile_pool(name="sup", bufs=2))
        pt = ctx.enter_context(tc.tile_pool(name="pt", bufs=8))
        acc = ctx.enter_context(tc.tile_pool(name="acc", bufs=1))
        psg = ctx.enter_context(tc.tile_pool(name="psg", bufs=2, space="PSUM"))
        pss = ctx.enter_context(tc.tile_pool(name="pss", bufs=2, space="PSUM"))
        psw = ctx.enter_context(tc.tile_pool(name="psw", bufs=2, space="PSUM"))

        # ---- constants (emitted below, after the first transposes, so the
        # SP queue reaches the first-supertile transposes immediately; the
        # GEMM-blocking acat8 rides the startup-idle ACT queue) ----
        acat8 = const.tile([P, KC2, 2, APAD], F8)
        w2c8 = const.tile([P, KC2, 2, 2], F8)
        c10 = const.tile([10, P], F16)
        onesc = const.tile([P, 1], F16)
        onehot4 = const.tile([P, ST, 8], F16)
        vs_all = const.tile([10, TILES * P], F16)
        vc_all = const.tile([10, TILES * P], F16)

        # ---- per-core accumulators ----
        hp_all = acc.tile([P, KC, RPB], F16)   # pooled head, feature-major
        tp_all = acc.tile([P, KC, RPB], F16)   # pooled tail, feature-major

        # loads are batched per PAIR of supertiles: the SWDGE descriptor-gen
        # time on the Pool engine is ~1-2.4us per instruction regardless of
        # size, and 4 cast-loads/supertile made Pool the DMA-issue serializer
        SG = 2 * ST
        GSUP = SG // ST
        head_r = head_d.rearrange("(g t p) h -> g p t h", t=SG, p=P)
        tail8_r = tail8_d.rearrange("(g t p) h -> g p t h", t=SG, p=P)
        taillo_r = taillo_d.rearrange("(g t p) h -> g p t h", t=SG, p=P)

        loaded16 = {}
        loaded8 = {}

        def emit_loads16(g):
            # fp8 LO residual of tail (host-split): tail = t8 + t8lo to
            # ~2^-8 relative, which the TAIL value pools need (tpool lands
            # raw in the output; single fp8 there costs 3.5e-2 rel err).
            # The HEAD value pools read the fp8 h8 copy alone -- hpool's
            # quantization noise washes through the Wsv^T projection.
            t8lo = sup.tile([P, SG, H], F8, tag="t8lo", name=f"t8lo_{g}",
                            bufs=3)
            nc.gpsimd.dma_start(out=t8lo[:], in_=taillo_r[g])
            loaded16[g] = t8lo

        def emit_loads8(g):
            # fp8 copies feed the score-side GEMMs; h8 cast from the f32
            # rows by SWDGE, t8 pre-cast on host (so the lo residual pairs
            # exactly)
            h8 = sup.tile([P, SG, H], F8, tag="h8", name=f"h8_{g}", bufs=3)
            t8 = sup.tile([P, SG, H], F8, tag="t8", name=f"t8_{g}", bufs=3)
            nc.gpsimd.dma_start(out=h8[:], in_=head_r[g])
            nc.gpsimd.dma_start(out=t8[:], in_=tail8_r[g])
            loaded8[g] = (h8, t8)

        transposed = {}

        def emit_transpose(s):
            # xbar transpose to feature-major with fp8 PAIRS packed in uint16
            # -- ONE instruction per tensor per supertile. The whole-supertile
            # 2D transpose lands t-major: hT[p, t*KC2+c, n] = pair (2p,2p+1)
            # of feature chunk c, tile t, row n.
            h8, t8 = loaded8[s // GSUP]
            off = ST * (s % GSUP)
            hT = sup.tile([P, ST * KC2, P], U16, tag="hT", name=f"hT{s}", bufs=3)
            tT = sup.tile([P, ST * KC2, P], U16, tag="tT", name=f"tT{s}", bufs=3)
            nc.sync.dma_start_transpose(
                hT[:], h8[:, off:off + ST, :].bitcast(U16).rearrange(
                    "p t m -> p (t m)"))
            nc.sync.dma_start_transpose(
                tT[:], t8[:, off:off + ST, :].bitcast(U16).rearrange(
                    "p t m -> p (t m)"))
            transposed[s] = (hT, tT)

        def f8v(tT_, t, cc):
            # DoubleRow moving view of one tile's pair-packed chunk:
            # [K=128, 2, n=128]
            return tT_[:, t * KC2 + cc, :].bitcast(F8).rearrange(
                "p (n two) -> p two n", two=2)

        def f8g(tT_, cc):
            # DoubleRow moving view of chunk cc across the whole supertile:
            # [K=128, 2, t, n] (4D AP; rows grouped per tile)
            return tT_.rearrange("p (t c) n -> p t c n", c=KC2)[
                :, :, cc, :].bitcast(F8).rearrange(
                "p t (n two) -> p two t n", two=2)

        out_sb = acc.tile([P, BC // P, H], F32)
        out_r = out_d.rearrange("(r p) h -> p r h", p=P)

        def emit_final(half):
            # out[batch, :] = hpool @ Wsv^T + tpool, computed ROW-major:
            # hp_all chunks are the (free) stationary with batches on the
            # free axis, wsvT moving; tpool is transpose-accumulated into
            # the same PSUM group via an identity moving operand. One batched
            # ACT copy per PSUM bank evacuates, then DMA-store.
            for r in range(half * (BC // P // 2), (half + 1) * (BC // P // 2)):
                pA = psg.tile([P, GN], F32, tag="pg", name=f"pfA{r}")
                pB = psg.tile([P, GN], F32, tag="pg", name=f"pfB{r}")
                for j in range(KC):
                    tgt = pA[:, ts(j, P)] if j < 4 else pB[:, ts(j - 4, P)]
                    for c in range(KC):
                        nc.tensor.matmul(tgt, hp_all[:, c, ts(r, P)],
                                         wsvT[:, c, ts(j, P)],
                                         start=(c == 0), stop=False)
                    nc.tensor.matmul(tgt, tp_all[:, j, ts(r, P)], identh[:],
                                     start=False, stop=True)
                nc.scalar.copy(out_sb[:, r, 0:GN], pA[:])
                nc.scalar.copy(out_sb[:, r, GN:H], pB[:, :H - GN])
                nc.sync.dma_start(out=out_r[:, r, :], in_=out_sb[:, r, :])

        emit_loads16(0)
        emit_loads8(0)
        # acat8 (the GEMM-0 blocker) and w2c8 ride the ACT queue, which is
        # idle at startup, so the SP queue can issue supertile-0 transposes
        # with no constants ahead of them.
        nc.scalar.dma_start(out=acat8[:],
                            in_=acat_d.rearrange("c p i m -> p c i m"))
        nc.scalar.dma_start(out=w2c8[:],
                            in_=w2_d.rearrange("c p i m -> p c i m"))
        emit_transpose(0)
        nc.sync.dma_start(out=c10[:], in_=c10_d[:, :])
        nc.sync.dma_start(out=onesc[:], in_=onesc_d[:, :])
        nc.sync.dma_start(out=onehot4[:], in_=onehot_d.rearrange(
            "p (t e) -> p t e", e=8))
        nc.sync.dma_start(out=vs_all[:], in_=vs_d[:, :])
        nc.sync.dma_start(out=vc_all[:], in_=vc_d[:, :])
        wsvT = const.tile([P, KC, H], F16)
        nc.sync.dma_start(out=wsvT[:], in_=wsvT_d.rearrange("c p m -> p c m"))
        mw_all = const.tile([P, TILES], F32)
        nc.sync.dma_start(out=mw_all[:], in_=mw_d[:, :])
        identh = const.tile([P, P], F16)
        nc.sync.dma_start(out=identh[:], in_=identh_d[:, :])
        c0 = const.tile([P, 1], F32)
        nc.sync.dma_start(out=c0[:], in_=c0_d[:, :])
        hA8s = {}

        def emit_gemm(s):
            # -- big GEMM: hA = head @ [A_s | A_c], feature-major, fp8 DR --
            hT, tT = transposed[s]
            hA8 = sup.tile([P, FO, SN], F8, tag="hA8", name=f"hA8_{s}")
            for j in range(FO):
                for hh in range(SN // GN):
                    pg = psg.tile([P, GN], F32, tag="pg")
                    for tt in range(ST):
                        for cc in range(KC2):
                            nc.tensor.matmul(pg[:, ts(tt, P)],
                                             acat8[:, cc, :, ts(j, P)],
                                             f8v(hT, tt, cc),
                                             start=(cc == 0),
                                             stop=(cc == KC2 - 1),
                                             perf_mode=DR)
                    if j < 7:
                        nc.scalar.copy(hA8[:, j, ts(hh, GN)], pg[:])
                    else:
                        nc.vector.tensor_copy(hA8[:, j, ts(hh, GN)], pg[:])
            hA8s[s] = hA8

        def emit_tiles(s_idx):
            t8log = loaded16[s_idx // GSUP]
            h8g, t8g = loaded8[s_idx // GSUP]
            voff = ST * (s_idx % GSUP)
            hT, tT = transposed.pop(s_idx)
            hA8 = hA8s.pop(s_idx)
            h16 = h8g[:, voff:voff + ST, :]
            t16 = t8g[:, voff:voff + ST, :]
            t16lo = t8log[:, voff:voff + ST, :]

            # One quad = the 4 tiles of this supertile. All [P,1]-sized gate
            # and copy work is batched across the quad to amortize per-inst
            # overhead; wp4 strides each tile's workspace by 256 f32 so no
            # matmul output crosses a PSUM bank boundary.
            # wp4[:, t, :] cols: 0:48 ps_hp, 48:96 ps_tp, 96 ws_s, 97 ws_c,
            # 98 gs, 99 gc, 100 S*den_s, 101 S*den_c, 102 hv, 103 tv,
            # 104:232 e^T (fp16 x256 via bitcast), 232:256 pad.
            tg0 = s_idx * ST
            wp4 = psw.tile([P, ST, 256], F32, tag="wp")
            ps2s = [pss.tile([P, 2, 2, P], F32, tag="ps", name=f"ps{s_idx}_{h}")
                    for h in range(2)]

            for t in range(ST):
                tg = tg0 + t
                # -- packed scores (8 batches x 16x16) + masks (one rank-10
                # matmul: rows 0-8 cross-batch block mask, row 9 key mask) --
                ps_pair = ps2s[t // 2][:, t % 2]
                ps_s = ps_pair[:, 0, :]
                ps_c = ps_pair[:, 1, :]
                for cc in range(KC2):
                    nc.tensor.matmul(ps_s, hA8[:, 2 * cc:2 * cc + 2, ts(t, P)],
                                     f8v(hT, t, cc),
                                     start=(cc == 0), stop=False, perf_mode=DR)
                nc.tensor.matmul(ps_s, c10[:], vs_all[:, ts(tg, P)],
                                 start=False, stop=True)
                for cc in range(KC2):
                    nc.tensor.matmul(ps_c,
                                     hA8[:, KC + 2 * cc:KC + 2 * cc + 2, ts(t, P)],
                                     f8v(tT, t, cc),
                                     start=(cc == 0), stop=False, perf_mode=DR)
                nc.tensor.matmul(ps_c, c10[:], vc_all[:, ts(tg, P)],
                                 start=False, stop=True)

                # -- gate dot inputs: hv = head@u, tv = tail@w2 (key-major,
                # 1-column DoubleRow matmuls, ~free on the PE); u and w2 are
                # fp8 hi+lo pairs (the gate is the precision-dominant path) --
                for k in range(4 * KC2):
                    cc, i, r = k // 4, (k // 2) % 2, k % 2
                    nc.tensor.matmul(wp4[:, t, 102:103],
                                     f8v(hT, t, cc)[:, i, :],
                                     acat8[:, cc, i, 1536 + r:1537 + r],
                                     start=(k == 0), stop=(k == 4 * KC2 - 1))
                for k in range(4 * KC2):
                    cc, i, r = k // 4, (k // 2) % 2, k % 2
                    nc.tensor.matmul(wp4[:, t, 103:104],
                                     f8v(tT, t, cc)[:, i, :],
                                     w2c8[:, cc, i, r:r + 1],
                                     start=(k == 0), stop=(k == 4 * KC2 - 1))

            # -- softmax numerators (free axis), one exp per 2 tiles;
            # masked lanes are ~-2e3 after the 1/S exp scale --
            e2s = []
            for h in range(2):
                e2 = pt.tile([P, 2, 2, P], F16, tag=f"e2_{h}", bufs=3)
                nc.scalar.activation(out=e2[:], in_=ps2s[h][:], func=Exp,
                                     bias=0.0, scale=1.0 / S)
                e2s.append(e2)

            # -- e^T on the PE so den/gs become 1-column matmuls --
            for t in range(ST):
                eT = wp4[:, t, 104:232].bitcast(F16).rearrange(
                    "p (c n) -> p c n", c=2)
                e_pair = e2s[t // 2][:, t % 2]
                nc.tensor.transpose(eT[:, 0, :], e_pair[:, 0, :], identh[:])
                nc.tensor.transpose(eT[:, 1, :], e_pair[:, 1, :], identh[:])

            # -- batched PSUM->SBUF staging for the whole quad --
            esT4 = pt.tile([P, ST, 2, P], F16, tag="esT", bufs=2)
            nc.vector.tensor_copy(
                esT4[:], wp4[:, :, 104:232].bitcast(F16).rearrange(
                    "p t (c n) -> p t c n", c=2))
            hvtv4 = pt.tile([P, ST, 2], F16, tag="hvtv", bufs=2)
            nc.vector.tensor_copy(hvtv4[:], wp4[:, :, 102:104])

            # cols: 98 gs_num, 99 gc_num, 100 S*den_s, 101 S*den_c
            for t in range(ST):
                nc.tensor.matmul(wp4[:, t, 98:99], esT4[:, t, 0, :],
                                 hvtv4[:, t, 0:1], start=True, stop=True)
                nc.tensor.matmul(wp4[:, t, 99:100], esT4[:, t, 1, :],
                                 hvtv4[:, t, 1:2], start=True, stop=True)
                nc.tensor.matmul(wp4[:, t, 100:101], esT4[:, t, 0, :],
                                 onesc[:], start=True, stop=True)
                nc.tensor.matmul(wp4[:, t, 101:102], esT4[:, t, 1, :],
                                 onesc[:], start=True, stop=True)

            # -- batched gate math (gate-as-exp; sign flips ride the host
            # constants onesc=-S, mw=-S*mw, c0=-C0 so the ACT engine only
            # ever needs the Exp table) --
            rden4 = pt.tile([P, ST, 2], F32, tag="rden", bufs=2)
            nc.vector.reciprocal(out=rden4[:], in_=wp4[:, :, 100:102])
            m4 = pt.tile([P, ST, 2], F32, tag="m4", bufs=2)
            nc.vector.tensor_mul(out=m4[:], in0=wp4[:, :, 98:100], in1=rden4[:])
            garg4 = pt.tile([P, ST], F32, tag="garg", bufs=2)
            nc.vector.tensor_add(out=garg4[:], in0=m4[:, :, 0], in1=m4[:, :, 1])
            eg4 = pt.tile([P, ST], F32, tag="eg", bufs=2)
            nc.scalar.activation(out=eg4[:], in_=garg4[:], func=Exp,
                                 bias=c0[:, 0:1], scale=1.0)
            gp4 = pt.tile([P, ST], F32, tag="gp", bufs=2)
            nc.vector.tensor_scalar_add(out=gp4[:], in0=eg4[:], scalar1=1.0)
            gate4 = pt.tile([P, ST], F32, tag="gate", bufs=2)
            nc.vector.reciprocal(out=gate4[:], in_=gp4[:])

            # -- pooling coefficient vectors (fold S*mw and 1/(S den)) --
            mw4 = mw_all[:, tg0:tg0 + ST]
            mwg4 = pt.tile([P, ST], F16, tag="mwg", bufs=2)   # S*mw*gate
            nc.vector.tensor_mul(out=mwg4[:], in0=mw4, in1=gate4[:])
            a_s4 = pt.tile([P, ST], F16, tag="a_s", bufs=2)
            nc.vector.tensor_mul(out=a_s4[:], in0=mwg4[:], in1=rden4[:, :, 0])
            mwc4 = pt.tile([P, ST], F16, tag="mwc", bufs=2)   # S*mw*(1-gate)
            nc.vector.tensor_sub(out=mwc4[:], in0=mw4, in1=mwg4[:])
            a_c4 = pt.tile([P, ST], F16, tag="a_c", bufs=2)
            nc.vector.tensor_mul(out=a_c4[:], in0=mwc4[:], in1=rden4[:, :, 1])

            # -- ws = e^T @ a : per-key pooled weights (block-diag safe) --
            for t in range(ST):
                e_pair = e2s[t // 2][:, t % 2]
                nc.tensor.matmul(wp4[:, t, 96:97], e_pair[:, 0, :],
                                 a_s4[:, t:t + 1], start=True, stop=True)
                nc.tensor.matmul(wp4[:, t, 97:98], e_pair[:, 1, :],
                                 a_c4[:, t:t + 1], start=True, stop=True)

            # -- block-diagonal weight columns via onehot, whole quad --
            diag_s4 = pt.tile([P, ST, 8], F16, tag="diag_s", bufs=2)
            diag_c4 = pt.tile([P, ST, 8], F16, tag="diag_c", bufs=2)
            nc.vector.tensor_tensor(out=diag_s4[:],
                                    in0=wp4[:, :, 96:97].to_broadcast([P, ST, 8]),
                                    in1=onehot4[:], op=mult)
            nc.vector.tensor_tensor(out=diag_c4[:],
                                    in0=wp4[:, :, 97:98].to_broadcast([P, ST, 8]),
                                    in1=onehot4[:], op=mult)

            # -- pools: feature-major pooled vectors, 8 batches per tile --
            for t in range(ST):
                ps_hp = wp4[:, t, 0:48].rearrange("p (c e) -> p c e", e=8)
                ps_tp = wp4[:, t, 48:96].rearrange("p (c e) -> p c e", e=8)
                for c in range(KC):
                    nc.tensor.matmul(ps_hp[:, c, :], h16[:, t, ts(c, P)],
                                     diag_s4[:, t, :], start=True, stop=True)
                    nc.tensor.matmul(ps_tp[:, c, :], t16[:, t, ts(c, P)],
                                     diag_c4[:, t, :], start=True, stop=False)
                    nc.tensor.matmul(ps_tp[:, c, :], t16lo[:, t, ts(c, P)],
                                     diag_c4[:, t, :], start=False, stop=True)
            nc.vector.tensor_copy(
                hp_all[:, :, tg0 * 8:(tg0 + ST) * 8].rearrange(
                    "p c (t e) -> p t c e", e=8),
                wp4[:, :, 0:48].rearrange("p t (c e) -> p t c e", e=8))
            nc.vector.tensor_copy(
                tp_all[:, :, tg0 * 8:(tg0 + ST) * 8].rearrange(
                    "p c (t e) -> p t c e", e=8),
                wp4[:, :, 48:96].rearrange("p t (c e) -> p t c e", e=8))

        for s_idx in range(NSUP):
            if s_idx == NSUP // 2:
                emit_final(0)
            if s_idx % GSUP == 0 and s_idx // GSUP + 1 < NSUP // GSUP:
                emit_loads16(s_idx // GSUP + 1)
                emit_loads8(s_idx // GSUP + 1)
            if s_idx + 1 < NSUP:
                emit_transpose(s_idx + 1)
            emit_gemm(s_idx)
            emit_tiles(s_idx)

        emit_final(1)


_NC_CACHE = None


def _get_nc():
    global _NC_CACHE
    if _NC_CACHE is None:
        _NC_CACHE = _build_nc()
    return _NC_CACHE


def _host_prep(Wsq, Wsk, Wsv, Wcq, Wck, Wg, bg, bsv,
               head_mask, tail_mask):
    """Fold weights; build per-core constant tensors (shared across cores
    except the mask-derived ones)."""
    f64 = np.float64
    scale = 1.0 / np.sqrt(f64(H))
    A_s = (Wsq.astype(f64).T @ Wsk.astype(f64)) * scale
    A_c = (Wcq.astype(f64).T @ Wck.astype(f64)) * scale
    A = np.concatenate([A_s, A_c], axis=1)                         # [768, 1536]
    # per 256-block of output features: even columns then odd columns, so
    # the big GEMM's PSUM chunk pairs (2c, 2c+1) hold features 256c+2p+i
    colperm = np.concatenate([
        np.concatenate([np.arange(256 * b, 256 * b + 256, 2),
                        np.arange(256 * b + 1, 256 * b + 256, 2)])
        for b in range(2 * H // 256)])
    Wg1 = Wg[0, :H].astype(f64)
    w2 = Wg[0, H:].astype(f64)
    u = Wsv.astype(f64).T @ Wg1
    uS = S * u
    u_hi = (uS.astype(np.float32)).astype(NP8)
    u_lo = uS - u_hi.astype(f64)                  # quantized again by the cast
    w2S = S * w2
    w2_hi = (w2S.astype(np.float32)).astype(NP8)
    w2_lo = w2S - w2_hi.astype(f64)
    acat = np.concatenate([A[:, colperm], u_hi.astype(f64)[:, None] / S,
                           u_lo[:, None] / S,
                           np.zeros((H, APAD - ACOLS - 1))], axis=1)  # [768, 1552]
    # rows (input features) interleaved: acat8[c, p, i] = S*acat[256c+2p+i]
    acat8 = (S * acat).reshape(KC2, P, 2, APAD).astype(NP8)
    w2_8 = np.stack([w2_hi.astype(f64), w2_lo], axis=-1)
    w2_8 = (w2_8).reshape(KC2, P, 2, 2).astype(NP8)
    wsvT_t = Wsv.astype(f64).T.reshape(KC, P, H).astype(np.float16)

    g = np.arange(P) // M                                          # group id per row
    # rank-10 combined mask operand: rows 0-8 = cross-batch block mask
    # (NEG everywhere, un-NEG within own 16-row block), row 9 = key mask
    c10 = np.zeros((10, P), np.float16)
    c9r = np.zeros((9, P), np.float16)
    c10[0] = 1.0
    c9r[0] = NEG
    for k in range(8):
        c10[1 + k] = (g == k).astype(np.float16)
        c9r[1 + k] = -NEG * (g == k).astype(np.float16)
    c10[9] = 1.0
    onesc = np.full((P, 1), -S, np.float16)   # negative: see gate-as-exp note
    onehot = np.zeros((P, 8), np.float16)
    onehot[np.arange(P), g] = 1.0
    onehot = np.tile(onehot, (1, 4))          # one copy per quad tile

    C0 = float(bg[0] + f64(bsv) @ Wg1)
    c0 = np.full((P, 1), -C0, np.float32)     # negated: gate-as-exp
    identh = np.eye(P, dtype=np.float16)

    # per-core mask-derived tensors: [10, TILES*P] moving operands whose
    # rows 0-8 repeat c9r every tile and row 9 carries the key mask
    hm = head_mask.reshape(NCORES, BC, M)
    tm = tail_mask.reshape(NCORES, BC, N)
    c9r_t = np.tile(c9r[:, None, :], (1, TILES, 1)).reshape(9, TILES * P)
    vs, vc, mw = [], [], []
    for i in range(NCORES):
        vsi = ((1 - hm[i]).astype(np.float16) * np.float16(NEG)
               ).reshape(1, TILES * P)
        vci = ((1 - tm[i]).astype(np.float16) * np.float16(NEG)
               ).reshape(1, TILES * P)
        vs.append(np.concatenate([c9r_t, vsi], axis=0))
        vc.append(np.concatenate([c9r_t, vci], axis=0))
        e = np.exp(hm[i].astype(f64))
        mwi = (-S * e / e.sum(axis=1, keepdims=True)).astype(np.float32)
        mw.append(mwi.reshape(TILES, P).T.copy())                    # [P, TILES]
    shared = dict(acat=acat8, w2c=w2_8, wsvT=wsvT_t, c10=c10,
                  onesc=onesc, onehot=onehot,
                  identh=identh, c0=c0)
    return shared, vs, vc, mw


def _core_feeds(head_mentions, tail_mentions, shared, vs, vc, mw, i):
    hm = head_mentions.reshape(NCORES, ROWS, H)
    tm = tail_mentions.reshape(NCORES, ROWS, H)
    t = np.ascontiguousarray(tm[i])
    t8 = t.astype(NP8)                       # hi fp8 (host cast, exact pair)
    tlo = t - t8.astype(np.float32)          # residual, SWDGE-cast to fp8
    feeds = {"head": np.ascontiguousarray(hm[i]),
             "tail8": t8, "taillo": tlo,
             "vs": vs[i], "vc": vc[i], "mw": mw[i]}
    feeds.update(shared)
    return feeds


def _reference_numpy(head_mentions, tail_mentions, head_mask, tail_mask,
                     Wsq, bsq, Wsk, bsk, Wsv, bsv, Wcq, bcq, Wck, bck, Wg, bg):
    """Exact fallback (only used if projection biases are nonzero)."""
    f = np.float32
    scale = f(1.0) / np.sqrt(f(H))
    hm = head_mentions.astype(f)
    tm = tail_mentions.astype(f)
    sq = hm @ Wsq.T + bsq
    sk = hm @ Wsk.T + bsk
    sv = hm @ Wsv.T + bsv
    ss = np.einsum("bmh,bnh->bmn", sq, sk) * scale
    ss = np.where(head_mask[:, None, :] == 0, f(NEG), ss)
    ss = ss - ss.max(-1, keepdims=True)
    e = np.exp(ss)
    sw = e / e.sum(-1, keepdims=True)
    self_out = np.einsum("bmn,bnh->bmh", sw, sv)
    cq = hm @ Wcq.T + bcq
    ck = tm @ Wck.T + bck
    cs = np.einsum("bmh,bnh->bmn", cq, ck) * scale
    cs = np.where(tail_mask[:, None, :] == 0, f(NEG), cs)
    cs = cs - cs.max(-1, keepdims=True)
    ec = np.exp(cs)
    cw = ec / ec.sum(-1, keepdims=True)
    cross_out = np.einsum("bmn,bnh->bmh", cw, tm)
    gate_in = np.concatenate([self_out, cross_out], axis=-1)
    gate = 1.0 / (1.0 + np.exp(-(np.einsum("bmh,oh->bmo", gate_in, Wg) + bg)))
    fused = gate * self_out + (1 - gate) * cross_out
    mexp = np.exp(head_mask.astype(f))
    mw = (mexp / mexp.sum(1, keepdims=True))[:, :, None]
    return (fused * mw).sum(axis=1)


def kernel(head_mentions, tail_mentions, head_mask, tail_mask,
           Wsq, bsq, Wsk, bsk, Wsv, bsv, Wcq, bcq, Wck, bck, Wg, bg,
           _trace=False):
    head_mentions = np.asarray(head_mentions)
    tail_mentions = np.asarray(tail_mentions)
    head_mask = np.asarray(head_mask)
    tail_mask = np.asarray(tail_mask)
    args = dict(Wsq=np.asarray(Wsq), bsq=np.asarray(bsq), Wsk=np.asarray(Wsk),
                bsk=np.asarray(bsk), Wsv=np.asarray(Wsv), bsv=np.asarray(bsv),
                Wcq=np.asarray(Wcq), bcq=np.asarray(bcq), Wck=np.asarray(Wck),
                bck=np.asarray(bck), Wg=np.asarray(Wg), bg=np.asarray(bg))

    # The folded formulation absorbs bg/bsv exactly; nonzero Q/K-side biases
    # (never produced by this problem's setup) would change the softmax and
    # are handled by the exact numpy fallback.
    if any(np.any(args[k] != 0) for k in ("bsq", "bsk", "bcq", "bck")):
        return _reference_numpy(head_mentions, tail_mentions, head_mask,
                                tail_mask, **args).astype(np.float32)

    shared, vs, vc, mw = _host_prep(args["Wsq"], args["Wsk"], args["Wsv"],
                                    args["Wcq"], args["Wck"], args["Wg"],
                                    args["bg"], args["bsv"],
                                    head_mask, tail_mask)

    nc = _get_nc()
    in_maps = [_core_feeds(head_mentions, tail_mentions, shared, vs, vc, mw, i)
               for i in range(NCORES)]
    res = run_bass_kernel_spmd(nc, in_maps, core_ids=list(range(NCORES)),
                               trace=_trace)
    out = np.concatenate([res.results[i]["out"] for i in range(NCORES)], axis=0)
    if _trace:
        kernel._last_result = res
    return out.astype(np.float32)



# revision 33
# speedup vs baseline: 1.2251x; 1.1054x over previous
"""Trainium2 Bass kernel for nn_EntityMentionAggregation.

Reference computation (per batch b, M=N=16 mentions, H=768):
  self-attn over head mentions, cross-attn head->tail, sigmoid-gated fusion,
  mask-softmax pooling over mentions -> out [B, H].

Algebraic restructuring (exact, given the zero biases produced by
setup_inputs; nonzero projection biases fall back to numpy):
  s_scores = scale * head @ (Wsq^T Wsk) @ head^T          (A_s folded)
  c_scores = scale * head @ (Wcq^T Wck) @ tail^T          (A_c folded)
  out      = hpool @ Wsv^T + tpool
    hpool  = ws_s^T-weighted sum of head rows, ws_s = s_w^T (mw*gate/den_s)
    tpool  = ws_c^T-weighted sum of tail rows
  gate     = sigmoid(s_w@(head@u) + c_w@(tail@w2) + C0), u = Wsv^T Wg1
so the V projection runs on pooled vectors (16x fewer rows) and
self_out/cross_out are never materialized.

Precision split: the score path (big GEMM + packed per-tile attention
matmuls + gate dot-products) runs in fp8 e4m3 with DoubleRow perf mode
(2 k-tiles of 128 per matmul at 0.5 cyc/row). On the value path the TAIL
pools stay fp16 (tpool lands raw in the output; fp8 there costs 3.5e-2
rel err) while the HEAD pools reuse the fp8 copy (hpool's quantization
noise washes through the Wsv^T projection; 1.35e-2 total, under the
2e-2 gate), which drops the fp16 head load entirely. The fp8 operands are
produced by a second SWDGE cast-load (f32->fp8) and transposed to
feature-major via the SBUF xbar with PAIRS of fp8 values packed in one
uint16 element; the resulting [feat-pair partition, 2, row] layout is
exactly DoubleRow's expected [K,2,N] k-tile shape (logical feature
f = 256c + 2p + i).  The folded A matrix is stored column-permuted
(per 256-block: even columns then odd columns) so the big GEMM's PSUM
partitions line up with the same pairing when its output chunks are used
as score-matmul weights.

Gate path: e = exp(scores/S) is transposed on the PE (identity matmul) so
den = e^T @ (-S*ones) and gs = e^T @ (head@u) become 1-column matmuls,
removing the partition-broadcast DRAM round-trip of hv entirely. The
sigmoid is evaluated as 1/(1+exp(-garg)) so the ACT engine only ever
needs the Exp table (Sigmoid lives in a different act-table set; each
switch would cost a 1.3us table reload); the sign flips ride host
constants (onesc=-S, mw=-S*mw, c0=-C0) and cancel in a_s/a_c. u and w2
are stored as fp8 hi+lo residual pairs - the gate dot-products were the
precision-dominant path (rel err 1.26e-2 -> 6.7e-3 on HW).
Scores carry a factor S=32 (folded into A) so the fp8 tensors sit inside
e4m3's dynamic range (max 240); exp applies scale 1/S.

Layout: batch is sharded 8 ways (512 batches/core); rows are processed in
tiles of 128 = 8 batches x 16 mentions. Cross-batch blocks are masked to
-inf via a rank-9 constant matmul and the per-tile key-mask via a rank-1
matmul, so softmax zeroes them exactly and the packed attention matrix is
block-diagonal -- which makes the pooling contractions plain matmuls
against block-diagonal weight columns built with a onehot multiply.
"""

import numpy as np
import ml_dtypes
import bass_rust
import concourse.bass as bass
import concourse.mybir as mybir
import concourse.tile as tile
from concourse.bass import ts
from concourse.bass_utils import run_bass_kernel_spmd

H = 768
B, M, N = 4096, 16, 16
NEG = -65504.0
P = 128
NCORES = 8
BC = B // NCORES          # batches per core = 512
ROWS = BC * M             # rows per core = 8192
TILES = ROWS // P         # 64 tiles (8 batches each)
ST = 4                    # tiles per supertile (GEMM moving N = 512)
NSUP = TILES // ST        # 16 supertiles
SN = ST * P               # 512 rows per supertile
GN = 512                  # GEMM moving width per PSUM pass (one bank)
KC = H // P               # 6 contraction chunks (128 each)
KC2 = H // (2 * P)        # 3 DoubleRow chunk pairs (256 each)
FO = 2 * H // P           # 12 score-feature chunks (A_s | A_c)
ACOLS = 2 * H + 1         # 1537: A_s | A_c | u
APAD = 1552               # ACOLS padded so the DoubleRow pair step is 16B-aligned
RPB = ROWS // M           # 512 pooled rows (batches) per core
S = 32.0                  # fp8 dynamic-range scale folded into A/u/w2/mw

F8 = mybir.dt.float8e4
F16 = mybir.dt.float16
F32 = mybir.dt.float32
U16 = mybir.dt.uint16
DR = mybir.MatmulPerfMode.DoubleRow
NP8 = ml_dtypes.float8_e4m3


def _split_sync_waits(nc):
    """Walrus caps sync waits per instruction (1 is the only universally
    accepted count in this toolchain). Hoist excess waits onto preceding
    single-wait EventSemaphore carriers on the same engine."""
    for f in nc.m.functions:
        for bb in f.blocks:
            il = bb.instructions
            new_il = []
            changed = False
            for inst in il:
                si = inst.sync_info
                if si is not None and len(si.on_wait) > 1:
                    waits = list(si.on_wait)
                    k = 0
                    while len(waits) > 1:
                        w, waits = waits[0], waits[1:]
                        d = bass_rust.InstEventSemaphore(
                            name=f"{inst.name}-wsplit{k}", ins=[], outs=[])
                        d.engine = inst.engine
                        d.sync_info = bass_rust.SyncInfo(on_wait=[w], on_update=[])
                        new_il.append(d)
                        k += 1
                        changed = True
                    inst.sync_info = bass_rust.SyncInfo(
                        on_wait=waits, on_update=list(si.on_update))
                new_il.append(inst)
            if changed:
                bb.instructions = new_il


def _build_nc(split=True):
    nc = bass.Bass(target_bir_lowering=False)

    head_d = nc.dram_tensor("head", [ROWS, H], F32, kind="ExternalInput")
    tail8_d = nc.dram_tensor("tail8", [ROWS, H], F8, kind="ExternalInput")
    taillo_d = nc.dram_tensor("taillo", [ROWS, H], F32, kind="ExternalInput")
    acat_d = nc.dram_tensor("acat", [KC2, P, 2, APAD], F8, kind="ExternalInput")
    w2_d = nc.dram_tensor("w2c", [KC2, P, 2, 2], F8, kind="ExternalInput")
    wsvT_d = nc.dram_tensor("wsvT", [KC, P, H], F16, kind="ExternalInput")
    c10_d = nc.dram_tensor("c10", [10, P], F16, kind="ExternalInput")
    onesc_d = nc.dram_tensor("onesc", [P, 1], F16, kind="ExternalInput")
    onehot_d = nc.dram_tensor("onehot", [P, ST * 8], F16, kind="ExternalInput")
    vs_d = nc.dram_tensor("vs", [10, TILES * P], F16, kind="ExternalInput")
    vc_d = nc.dram_tensor("vc", [10, TILES * P], F16, kind="ExternalInput")
    mw_d = nc.dram_tensor("mw", [P, TILES], F32, kind="ExternalInput")
    identh_d = nc.dram_tensor("identh", [P, P], F16, kind="ExternalInput")
    c0_d = nc.dram_tensor("c0", [P, 1], F32, kind="ExternalInput")
    out_d = nc.dram_tensor("out", [BC, H], F32, kind="ExternalOutput")

    with tile.TileContext(nc) as tc:
        _emit(nc, tc, head_d, tail8_d, taillo_d, acat_d, w2_d, wsvT_d, c10_d,
              onesc_d, onehot_d, vs_d, vc_d, mw_d,
              identh_d, c0_d, out_d)
    if split:
        _split_sync_waits(nc)
    return nc


def _emit(nc, tc, head_d, tail8_d, taillo_d, acat_d, w2_d, wsvT_d, c10_d,
          onesc_d, onehot_d, vs_d, vc_d, mw_d,
          identh_d, c0_d, out_d):
    from contextlib import ExitStack
    Exp = mybir.ActivationFunctionType.Exp
    Sig = mybir.ActivationFunctionType.Sigmoid
    Ident = mybir.ActivationFunctionType.Identity
    mult = mybir.AluOpType.mult
    ctx = ExitStack()
    with ctx:
        const = ctx.enter_context(tc.tile_pool(name="const", bufs=1))
        sup = ctx.enter_context(tc.tile_pool(name="sup", bufs=2))
        pt = ctx.enter_context(tc.tile_pool(name="pt", bufs=8))
        acc = ctx.enter_context(tc.tile_pool(name="acc", bufs=1))
        psg = ctx.enter_context(tc.tile_pool(name="psg", bufs=3, space="PSUM"))
        pss = ctx.enter_context(tc.tile_pool(name="pss", bufs=3, space="PSUM"))
        psw = ctx.enter_context(tc.tile_pool(name="psw", bufs=2, space="PSUM"))

        # ---- constants (emitted below, after the first transposes, so the
        # SP queue reaches the first-supertile transposes immediately; the
        # GEMM-blocking acat8 rides the startup-idle ACT queue) ----
        acat8 = const.tile([P, KC2, 2, APAD], F8)
        w2c8 = const.tile([P, KC2, 2, 2], F8)
        c10 = const.tile([10, P], F16)
        onesc = const.tile([P, 1], F16)
        onehot4 = const.tile([P, ST, 8], F16)
        vs_all = const.tile([10, TILES * P], F16)
        vc_all = const.tile([10, TILES * P], F16)

        # ---- per-core accumulators ----
        hp_all = acc.tile([P, KC, RPB], F16)   # pooled head, feature-major
        tp_all = acc.tile([P, KC, RPB], F16)   # pooled tail, feature-major

        # loads are batched per PAIR of supertiles: the SWDGE descriptor-gen
        # time on the Pool engine is ~1-2.4us per instruction regardless of
        # size, and 4 cast-loads/supertile made Pool the DMA-issue serializer
        SG = 2 * ST
        GSUP = SG // ST
        head_r = head_d.rearrange("(g t p) h -> g p t h", t=SG, p=P)
        tail8_r = tail8_d.rearrange("(g t p) h -> g p t h", t=SG, p=P)
        taillo_r = taillo_d.rearrange("(g t p) h -> g p t h", t=SG, p=P)

        loaded16 = {}
        loaded8 = {}

        def emit_loads16(g):
            # fp8 LO residual of tail (host-split): tail = t8 + t8lo to
            # ~2^-8 relative, which the TAIL value pools need (tpool lands
            # raw in the output; single fp8 there costs 3.5e-2 rel err).
            # The HEAD value pools read the fp8 h8 copy alone -- hpool's
            # quantization noise washes through the Wsv^T projection.
            t8lo = sup.tile([P, SG, H], F8, tag="t8lo", name=f"t8lo_{g}",
                            bufs=3)
            nc.gpsimd.dma_start(out=t8lo[:], in_=taillo_r[g])
            loaded16[g] = t8lo

        def emit_loads8(g):
            # fp8 copies feed the score-side GEMMs; h8 cast from the f32
            # rows by SWDGE, t8 pre-cast on host (so the lo residual pairs
            # exactly)
            h8 = sup.tile([P, SG, H], F8, tag="h8", name=f"h8_{g}", bufs=3)
            t8 = sup.tile([P, SG, H], F8, tag="t8", name=f"t8_{g}", bufs=3)
            nc.gpsimd.dma_start(out=h8[:], in_=head_r[g])
            nc.gpsimd.dma_start(out=t8[:], in_=tail8_r[g])
            loaded8[g] = (h8, t8)

        transposed = {}

        def emit_transpose(s):
            # xbar transpose to feature-major with fp8 PAIRS packed in uint16
            # -- ONE instruction per tensor per supertile. The whole-supertile
            # 2D transpose lands t-major: hT[p, t*KC2+c, n] = pair (2p,2p+1)
            # of feature chunk c, tile t, row n.
            h8, t8 = loaded8[s // GSUP]
            off = ST * (s % GSUP)
            hT = sup.tile([P, ST * KC2, P], U16, tag="hT", name=f"hT{s}", bufs=3)
            tT = sup.tile([P, ST * KC2, P], U16, tag="tT", name=f"tT{s}", bufs=3)
            nc.sync.dma_start_transpose(
                hT[:], h8[:, off:off + ST, :].bitcast(U16).rearrange(
                    "p t m -> p (t m)"))
            nc.sync.dma_start_transpose(
                tT[:], t8[:, off:off + ST, :].bitcast(U16).rearrange(
                    "p t m -> p (t m)"))
            transposed[s] = (hT, tT)

        def f8v(tT_, t, cc):
            # DoubleRow moving view of one tile's pair-packed chunk:
            # [K=128, 2, n=128]
            return tT_[:, t * KC2 + cc, :].bitcast(F8).rearrange(
                "p (n two) -> p two n", two=2)

        def f8g(tT_, cc):
            # DoubleRow moving view of chunk cc across the whole supertile:
            # [K=128, 2, t, n] (4D AP; rows grouped per tile)
            return tT_.rearrange("p (t c) n -> p t c n", c=KC2)[
                :, :, cc, :].bitcast(F8).rearrange(
                "p t (n two) -> p two t n", two=2)

        out_sb = acc.tile([P, BC // P, H], F32)
        out_r = out_d.rearrange("(r p) h -> p r h", p=P)

        def emit_final(r):
            # out[batch, :] = hpool @ Wsv^T + tpool, computed ROW-major:
            # hp_all chunks are the (free) stationary with batches on the
            # free axis, wsvT moving; tpool is transpose-accumulated into
            # the same PSUM group via an identity moving operand. One batched
            # ACT copy per PSUM bank evacuates, then DMA-store. Emitted one
            # 128-batch block at a time, spread across the supertile stream.
            if True:
                pA = psg.tile([P, GN], F32, tag="pg", name=f"pfA{r}")
                pB = psg.tile([P, GN], F32, tag="pg", name=f"pfB{r}")
                for j in range(KC):
                    tgt = pA[:, ts(j, P)] if j < 4 else pB[:, ts(j - 4, P)]
                    for c in range(KC):
                        nc.tensor.matmul(tgt, hp_all[:, c, ts(r, P)],
                                         wsvT[:, c, ts(j, P)],
                                         start=(c == 0), stop=False)
                    nc.tensor.matmul(tgt, tp_all[:, j, ts(r, P)], identh[:],
                                     start=False, stop=True)
                nc.scalar.copy(out_sb[:, r, 0:GN], pA[:])
                nc.scalar.copy(out_sb[:, r, GN:H], pB[:, :H - GN])
                nc.sync.dma_start(out=out_r[:, r, :], in_=out_sb[:, r, :])

        emit_loads16(0)
        emit_loads8(0)
        emit_transpose(0)
        # constants follow the supertile-0 transposes on SP so nothing
        # delays the first GEMM's inputs
        nc.sync.dma_start(out=acat8[:],
                          in_=acat_d.rearrange("c p i m -> p c i m"))
        nc.sync.dma_start(out=w2c8[:],
                          in_=w2_d.rearrange("c p i m -> p c i m"))
        nc.sync.dma_start(out=c10[:], in_=c10_d[:, :])
        nc.sync.dma_start(out=onesc[:], in_=onesc_d[:, :])
        nc.sync.dma_start(out=onehot4[:], in_=onehot_d.rearrange(
            "p (t e) -> p t e", e=8))
        nc.sync.dma_start(out=vs_all[:], in_=vs_d[:, :])
        nc.sync.dma_start(out=vc_all[:], in_=vc_d[:, :])
        wsvT = const.tile([P, KC, H], F16)
        nc.sync.dma_start(out=wsvT[:], in_=wsvT_d.rearrange("c p m -> p c m"))
        mw_all = const.tile([P, TILES], F32)
        nc.sync.dma_start(out=mw_all[:], in_=mw_d[:, :])
        identh = const.tile([P, P], F16)
        nc.sync.dma_start(out=identh[:], in_=identh_d[:, :])
        c0 = const.tile([P, 1], F32)
        nc.sync.dma_start(out=c0[:], in_=c0_d[:, :])
        hA8s = {}

        def emit_gemm(s):
            # -- big GEMM: hA = head @ [A_s | A_c], feature-major, fp8 DR --
            hT, tT = transposed[s]
            hA8 = sup.tile([P, FO, SN], F8, tag="hA8", name=f"hA8_{s}")
            for j in range(FO):
                for hh in range(SN // GN):
                    pg = psg.tile([P, GN], F32, tag="pg")
                    for tt in range(ST):
                        for cc in range(KC2):
                            nc.tensor.matmul(pg[:, ts(tt, P)],
                                             acat8[:, cc, :, ts(j, P)],
                                             f8v(hT, tt, cc),
                                             start=(cc == 0),
                                             stop=(cc == KC2 - 1),
                                             perf_mode=DR)
                    if j % 2 == 0 or j == 11:
                        nc.scalar.copy(hA8[:, j, ts(hh, GN)], pg[:])
                    else:
                        nc.vector.tensor_copy(hA8[:, j, ts(hh, GN)], pg[:])
            hA8s[s] = hA8

        def emit_tiles(s_idx):
            t8log = loaded16[s_idx // GSUP]
            h8g, t8g = loaded8[s_idx // GSUP]
            voff = ST * (s_idx % GSUP)
            hT, tT = transposed.pop(s_idx)
            hA8 = hA8s.pop(s_idx)
            h16 = h8g[:, voff:voff + ST, :]
            t16 = t8g[:, voff:voff + ST, :]
            t16lo = t8log[:, voff:voff + ST, :]

            # One quad = the 4 tiles of this supertile. All [P,1]-sized gate
            # and copy work is batched across the quad to amortize per-inst
            # overhead; wp4 strides each tile's workspace by 256 f32 so no
            # matmul output crosses a PSUM bank boundary.
            # wp4[:, t, :] cols: 0:48 ps_hp, 48:96 ps_tp, 96 ws_s, 97 ws_c,
            # 98 gs, 99 gc, 100 S*den_s, 101 S*den_c, 102 hv, 103 tv,
            # 104:232 e^T (fp16 x256 via bitcast), 232:256 pad.
            tg0 = s_idx * ST
            wp4 = psw.tile([P, ST, 128], F32, tag="wp")
            ps2s = [pss.tile([P, 2, 2, P], F32, tag="ps", name=f"ps{s_idx}_{h}")
                    for h in range(2)]
            eT4 = pss.tile([P, ST, 2, P], F16, tag="ps", name=f"eT{s_idx}")

            for t in range(ST):
                tg = tg0 + t
                # -- packed scores (8 batches x 16x16) + masks (one rank-10
                # matmul: rows 0-8 cross-batch block mask, row 9 key mask) --
                ps_pair = ps2s[t // 2][:, t % 2]
                ps_s = ps_pair[:, 0, :]
                ps_c = ps_pair[:, 1, :]
                for cc in range(KC2):
                    nc.tensor.matmul(ps_s, hA8[:, 2 * cc:2 * cc + 2, ts(t, P)],
                                     f8v(hT, t, cc),
                                     start=(cc == 0), stop=False, perf_mode=DR)
                nc.tensor.matmul(ps_s, c10[:], vs_all[:, ts(tg, P)],
                                 start=False, stop=True)
                for cc in range(KC2):
                    nc.tensor.matmul(ps_c,
                                     hA8[:, KC + 2 * cc:KC + 2 * cc + 2, ts(t, P)],
                                     f8v(tT, t, cc),
                                     start=(cc == 0), stop=False, perf_mode=DR)
                nc.tensor.matmul(ps_c, c10[:], vc_all[:, ts(tg, P)],
                                 start=False, stop=True)

                # -- gate dot inputs: hv = head@u, tv = tail@w2 (key-major,
                # 1-column DoubleRow matmuls, ~free on the PE); u and w2 are
                # fp8 hi+lo pairs (the gate is the precision-dominant path) --
                for k in range(4 * KC2):
                    cc, i, r = k // 4, (k // 2) % 2, k % 2
                    nc.tensor.matmul(wp4[:, t, 102:103],
                                     f8v(hT, t, cc)[:, i, :],
                                     acat8[:, cc, i, 1536 + r:1537 + r],
                                     start=(k == 0), stop=(k == 4 * KC2 - 1))
                for k in range(4 * KC2):
                    cc, i, r = k // 4, (k // 2) % 2, k % 2
                    nc.tensor.matmul(wp4[:, t, 103:104],
                                     f8v(tT, t, cc)[:, i, :],
                                     w2c8[:, cc, i, r:r + 1],
                                     start=(k == 0), stop=(k == 4 * KC2 - 1))

            # -- softmax numerators (free axis), one exp per 2 tiles;
            # masked lanes are ~-2e3 after the 1/S exp scale --
            e2s = []
            for h in range(2):
                e2 = pt.tile([P, 2, 2, P], F16, tag=f"e2_{h}", bufs=3)
                nc.scalar.activation(out=e2[:], in_=ps2s[h][:], func=Exp,
                                     bias=0.0, scale=1.0 / S)
                e2s.append(e2)

            # -- e^T on the PE so den/gs become 1-column matmuls --
            for t in range(ST):
                e_pair = e2s[t // 2][:, t % 2]
                nc.tensor.transpose(eT4[:, t, 0, :], e_pair[:, 0, :], identh[:])
                nc.tensor.transpose(eT4[:, t, 1, :], e_pair[:, 1, :], identh[:])

            # -- batched PSUM->SBUF staging for the whole quad --
            esT4 = pt.tile([P, ST, 2, P], F16, tag="esT", bufs=2)
            nc.vector.tensor_copy(esT4[:], eT4[:])
            hvtv4 = pt.tile([P, ST, 2], F16, tag="hvtv", bufs=2)
            nc.vector.tensor_copy(hvtv4[:], wp4[:, :, 102:104])

            # cols: 98 gs_num, 99 gc_num, 100 S*den_s, 101 S*den_c
            for t in range(ST):
                nc.tensor.matmul(wp4[:, t, 98:99], esT4[:, t, 0, :],
                                 hvtv4[:, t, 0:1], start=True, stop=True)
                nc.tensor.matmul(wp4[:, t, 99:100], esT4[:, t, 1, :],
                                 hvtv4[:, t, 1:2], start=True, stop=True)
                nc.tensor.matmul(wp4[:, t, 100:101], esT4[:, t, 0, :],
                                 onesc[:], start=True, stop=True)
                nc.tensor.matmul(wp4[:, t, 101:102], esT4[:, t, 1, :],
                                 onesc[:], start=True, stop=True)

            # -- batched gate math (gate-as-exp; sign flips ride the host
            # constants onesc=-S, mw=-S*mw, c0=-C0 so the ACT engine only
            # ever needs the Exp table) --
            rden4 = pt.tile([P, ST, 2], F32, tag="rden", bufs=2)
            nc.vector.reciprocal(out=rden4[:], in_=wp4[:, :, 100:102])
            m4 = pt.tile([P, ST, 2], F32, tag="m4", bufs=2)
            nc.vector.tensor_mul(out=m4[:], in0=wp4[:, :, 98:100], in1=rden4[:])
            garg4 = pt.tile([P, ST], F32, tag="garg", bufs=2)
            nc.vector.tensor_add(out=garg4[:], in0=m4[:, :, 0], in1=m4[:, :, 1])
            eg4 = pt.tile([P, ST], F32, tag="eg", bufs=2)
            nc.scalar.activation(out=eg4[:], in_=garg4[:], func=Exp,
                                 bias=c0[:, 0:1], scale=1.0)
            gp4 = pt.tile([P, ST], F32, tag="gp", bufs=2)
            nc.vector.tensor_scalar_add(out=gp4[:], in0=eg4[:], scalar1=1.0)
            gate4 = pt.tile([P, ST], F32, tag="gate", bufs=2)
            nc.vector.reciprocal(out=gate4[:], in_=gp4[:])

            # -- pooling coefficient vectors (fold S*mw and 1/(S den)) --
            mw4 = mw_all[:, tg0:tg0 + ST]
            mwg4 = pt.tile([P, ST], F16, tag="mwg", bufs=2)   # S*mw*gate
            nc.vector.tensor_mul(out=mwg4[:], in0=mw4, in1=gate4[:])
            a_s4 = pt.tile([P, ST], F16, tag="a_s", bufs=2)
            nc.vector.tensor_mul(out=a_s4[:], in0=mwg4[:], in1=rden4[:, :, 0])
            mwc4 = pt.tile([P, ST], F16, tag="mwc", bufs=2)   # S*mw*(1-gate)
            nc.vector.tensor_sub(out=mwc4[:], in0=mw4, in1=mwg4[:])
            a_c4 = pt.tile([P, ST], F16, tag="a_c", bufs=2)
            nc.vector.tensor_mul(out=a_c4[:], in0=mwc4[:], in1=rden4[:, :, 1])

            # -- ws = e^T @ a : per-key pooled weights (block-diag safe) --
            for t in range(ST):
                e_pair = e2s[t // 2][:, t % 2]
                nc.tensor.matmul(wp4[:, t, 96:97], e_pair[:, 0, :],
                                 a_s4[:, t:t + 1], start=True, stop=True)
                nc.tensor.matmul(wp4[:, t, 97:98], e_pair[:, 1, :],
                                 a_c4[:, t:t + 1], start=True, stop=True)

            # -- block-diagonal weight columns via onehot, whole quad --
            diag_s4 = pt.tile([P, ST, 8], F16, tag="diag_s", bufs=2)
            diag_c4 = pt.tile([P, ST, 8], F16, tag="diag_c", bufs=2)
            nc.vector.tensor_tensor(out=diag_s4[:],
                                    in0=wp4[:, :, 96:97].to_broadcast([P, ST, 8]),
                                    in1=onehot4[:], op=mult)
            nc.vector.tensor_tensor(out=diag_c4[:],
                                    in0=wp4[:, :, 97:98].to_broadcast([P, ST, 8]),
                                    in1=onehot4[:], op=mult)

            # -- pools: feature-major pooled vectors, 8 batches per tile --
            for t in range(ST):
                ps_hp = wp4[:, t, 0:48].rearrange("p (c e) -> p c e", e=8)
                ps_tp = wp4[:, t, 48:96].rearrange("p (c e) -> p c e", e=8)
                for c in range(KC):
                    nc.tensor.matmul(ps_hp[:, c, :], h16[:, t, ts(c, P)],
                                     diag_s4[:, t, :], start=True, stop=True)
                    nc.tensor.matmul(ps_tp[:, c, :], t16[:, t, ts(c, P)],
                                     diag_c4[:, t, :], start=True, stop=False)
                    nc.tensor.matmul(ps_tp[:, c, :], t16lo[:, t, ts(c, P)],
                                     diag_c4[:, t, :], start=False, stop=True)
            nc.vector.tensor_copy(
                hp_all[:, :, tg0 * 8:(tg0 + ST) * 8].rearrange(
                    "p c (t e) -> p t c e", e=8),
                wp4[:, :, 0:48].rearrange("p t (c e) -> p t c e", e=8))
            nc.vector.tensor_copy(
                tp_all[:, :, tg0 * 8:(tg0 + ST) * 8].rearrange(
                    "p c (t e) -> p t c e", e=8),
                wp4[:, :, 48:96].rearrange("p t (c e) -> p t c e", e=8))

        for s_idx in range(NSUP):
            if s_idx in (8, 10, 12):
                emit_final((s_idx - 8) // 2)
            if s_idx % GSUP == 0 and s_idx // GSUP + 1 < NSUP // GSUP:
                emit_loads16(s_idx // GSUP + 1)
                emit_loads8(s_idx // GSUP + 1)
            if s_idx + 1 < NSUP:
                emit_transpose(s_idx + 1)
            emit_gemm(s_idx)
            emit_tiles(s_idx)

        emit_final(3)


_NC_CACHE = None


def _get_nc():
    global _NC_CACHE
    if _NC_CACHE is None:
        _NC_CACHE = _build_nc()
    return _NC_CACHE


def _host_prep(Wsq, Wsk, Wsv, Wcq, Wck, Wg, bg, bsv,
               head_mask, tail_mask):
    """Fold weights; build per-core constant tensors (shared across cores
    except the mask-derived ones)."""
    f64 = np.float64
    scale = 1.0 / np.sqrt(f64(H))
    A_s = (Wsq.astype(f64).T @ Wsk.astype(f64)) * scale
    A_c = (Wcq.astype(f64).T @ Wck.astype(f64)) * scale
    A = np.concatenate([A_s, A_c], axis=1)                         # [768, 1536]
    # per 256-block of output features: even columns then odd columns, so
    # the big GEMM's PSUM chunk pairs (2c, 2c+1) hold features 256c+2p+i
    colperm = np.concatenate([
        np.concatenate([np.arange(256 * b, 256 * b + 256, 2),
                        np.arange(256 * b + 1, 256 * b + 256, 2)])
        for b in range(2 * H // 256)])
    Wg1 = Wg[0, :H].astype(f64)
    w2 = Wg[0, H:].astype(f64)
    u = Wsv.astype(f64).T @ Wg1
    uS = S * u
    u_hi = (uS.astype(np.float32)).astype(NP8)
    u_lo = uS - u_hi.astype(f64)                  # quantized again by the cast
    w2S = S * w2
    w2_hi = (w2S.astype(np.float32)).astype(NP8)
    w2_lo = w2S - w2_hi.astype(f64)
    acat = np.concatenate([A[:, colperm], u_hi.astype(f64)[:, None] / S,
                           u_lo[:, None] / S,
                           np.zeros((H, APAD - ACOLS - 1))], axis=1)  # [768, 1552]
    # rows (input features) interleaved: acat8[c, p, i] = S*acat[256c+2p+i]
    acat8 = (S * acat).reshape(KC2, P, 2, APAD).astype(NP8)
    w2_8 = np.stack([w2_hi.astype(f64), w2_lo], axis=-1)
    w2_8 = (w2_8).reshape(KC2, P, 2, 2).astype(NP8)
    wsvT_t = Wsv.astype(f64).T.reshape(KC, P, H).astype(np.float16)

    g = np.arange(P) // M                                          # group id per row
    # rank-10 combined mask operand: rows 0-8 = cross-batch block mask
    # (NEG everywhere, un-NEG within own 16-row block), row 9 = key mask
    c10 = np.zeros((10, P), np.float16)
    c9r = np.zeros((9, P), np.float16)
    c10[0] = 1.0
    c9r[0] = NEG
    for k in range(8):
        c10[1 + k] = (g == k).astype(np.float16)
        c9r[1 + k] = -NEG * (g == k).astype(np.float16)
    c10[9] = 1.0
    onesc = np.full((P, 1), -S, np.float16)   # negative: see gate-as-exp note
    onehot = np.zeros((P, 8), np.float16)
    onehot[np.arange(P), g] = 1.0
    onehot = np.tile(onehot, (1, 4))          # one copy per quad tile

    C0 = float(bg[0] + f64(bsv) @ Wg1)
    c0 = np.full((P, 1), -C0, np.float32)     # negated: gate-as-exp
    identh = np.eye(P, dtype=np.float16)

    # per-core mask-derived tensors: [10, TILES*P] moving operands whose
    # rows 0-8 repeat c9r every tile and row 9 carries the key mask
    hm = head_mask.reshape(NCORES, BC, M)
    tm = tail_mask.reshape(NCORES, BC, N)
    c9r_t = np.tile(c9r[:, None, :], (1, TILES, 1)).reshape(9, TILES * P)
    vs, vc, mw = [], [], []
    for i in range(NCORES):
        vsi = ((1 - hm[i]).astype(np.float16) * np.float16(NEG)
               ).reshape(1, TILES * P)
        vci = ((1 - tm[i]).astype(np.float16) * np.float16(NEG)
               ).reshape(1, TILES * P)
        vs.append(np.concatenate([c9r_t, vsi], axis=0))
        vc.append(np.concatenate([c9r_t, vci], axis=0))
        e = np.exp(hm[i].astype(f64))
        mwi = (-S * e / e.sum(axis=1, keepdims=True)).astype(np.float32)
        mw.append(mwi.reshape(TILES, P).T.copy())                    # [P, TILES]
    shared = dict(acat=acat8, w2c=w2_8, wsvT=wsvT_t, c10=c10,
                  onesc=onesc, onehot=onehot,
                  identh=identh, c0=c0)
    return shared, vs, vc, mw


def _core_feeds(head_mentions, tail_mentions, shared, vs, vc, mw, i):
    hm = head_mentions.reshape(NCORES, ROWS, H)
    tm = tail_mentions.reshape(NCORES, ROWS, H)
    t = np.ascontiguousarray(tm[i])
    t8 = t.astype(NP8)                       # hi fp8 (host cast, exact pair)
    tlo = t - t8.astype(np.float32)          # residual, SWDGE-cast to fp8
    feeds = {"head": np.ascontiguousarray(hm[i]),
             "tail8": t8, "taillo": tlo,
             "vs": vs[i], "vc": vc[i], "mw": mw[i]}
    feeds.update(shared)
    return feeds


def _reference_numpy(head_mentions, tail_mentions, head_mask, tail_mask,
                     Wsq, bsq, Wsk, bsk, Wsv, bsv, Wcq, bcq, Wck, bck, Wg, bg):
    """Exact fallback (only used if projection biases are nonzero)."""
    f = np.float32
    scale = f(1.0) / np.sqrt(f(H))
    hm = head_mentions.astype(f)
    tm = tail_mentions.astype(f)
    sq = hm @ Wsq.T + bsq
    sk = hm @ Wsk.T + bsk
    sv = hm @ Wsv.T + bsv
    ss = np.einsum("bmh,bnh->bmn", sq, sk) * scale
    ss = np.where(head_mask[:, None, :] == 0, f(NEG), ss)
    ss = ss - ss.max(-1, keepdims=True)
    e = np.exp(ss)
    sw = e / e.sum(-1, keepdims=True)
    self_out = np.einsum("bmn,bnh->bmh", sw, sv)
    cq = hm @ Wcq.T + bcq
    ck = tm @ Wck.T + bck
    cs = np.einsum("bmh,bnh->bmn", cq, ck) * scale
    cs = np.where(tail_mask[:, None, :] == 0, f(NEG), cs)
    cs = cs - cs.max(-1, keepdims=True)
    ec = np.exp(cs)
    cw = ec / ec.sum(-1, keepdims=True)
    cross_out = np.einsum("bmn,bnh->bmh", cw, tm)
    gate_in = np.concatenate([self_out, cross_out], axis=-1)
    gate = 1.0 / (1.0 + np.exp(-(np.einsum("bmh,oh->bmo", gate_in, Wg) + bg)))
    fused = gate * self_out + (1 - gate) * cross_out
    mexp = np.exp(head_mask.astype(f))
    mw = (mexp / mexp.sum(1, keepdims=True))[:, :, None]
    return (fused * mw).sum(axis=1)


def kernel(head_mentions, tail_mentions, head_mask, tail_mask,
           Wsq, bsq, Wsk, bsk, Wsv, bsv, Wcq, bcq, Wck, bck, Wg, bg,
           _trace=False):
    head_mentions = np.asarray(head_mentions)
    tail_mentions = np.asarray(tail_mentions)
    head_mask = np.asarray(head_mask)
    tail_mask = np.asarray(tail_mask)
    args = dict(Wsq=np.asarray(Wsq), bsq=np.asarray(bsq), Wsk=np.asarray(Wsk),
                bsk=np.asarray(bsk), Wsv=np.asarray(Wsv), bsv=np.asarray(bsv),
                Wcq=np.asarray(Wcq), bcq=np.asarray(bcq), Wck=np.asarray(Wck),
                bck=np.asarray(bck), Wg=np.asarray(Wg), bg=np.asarray(bg))

    # The folded formulation absorbs bg/bsv exactly; nonzero Q/K-side biases
    # (never produced by this problem's setup) would change the softmax and
    # are handled by the exact numpy fallback.
    if any(np.any(args[k] != 0) for k in ("bsq", "bsk", "bcq", "bck")):
        return _reference_numpy(head_mentions, tail_mentions, head_mask,
                                tail_mask, **args).astype(np.float32)

    shared, vs, vc, mw = _host_prep(args["Wsq"], args["Wsk"], args["Wsv"],
                                    args["Wcq"], args["Wck"], args["Wg"],
                                    args["bg"], args["bsv"],
                                    head_mask, tail_mask)

    nc = _get_nc()
    in_maps = [_core_feeds(head_mentions, tail_mentions, shared, vs, vc, mw, i)
               for i in range(NCORES)]
    res = run_bass_kernel_spmd(nc, in_maps, core_ids=list(range(NCORES)),
                               trace=_trace)
    out = np.concatenate([res.results[i]["out"] for i in range(NCORES)], axis=0)
    if _trace:
        kernel._last_result = res
    return out.astype(np.float32)



# revision 34
# speedup vs baseline: 1.3107x; 1.0699x over previous
"""Trainium2 Bass kernel for nn_EntityMentionAggregation.

Reference computation (per batch b, M=N=16 mentions, H=768):
  self-attn over head mentions, cross-attn head->tail, sigmoid-gated fusion,
  mask-softmax pooling over mentions -> out [B, H].

Algebraic restructuring (exact, given the zero biases produced by
setup_inputs; nonzero projection biases fall back to numpy):
  s_scores = scale * head @ (Wsq^T Wsk) @ head^T          (A_s folded)
  c_scores = scale * head @ (Wcq^T Wck) @ tail^T          (A_c folded)
  out      = hpool @ Wsv^T + tpool
    hpool  = ws_s^T-weighted sum of head rows, ws_s = s_w^T (mw*gate/den_s)
    tpool  = ws_c^T-weighted sum of tail rows
  gate     = sigmoid(s_w@(head@u) + c_w@(tail@w2) + C0), u = Wsv^T Wg1
so the V projection runs on pooled vectors (16x fewer rows) and
self_out/cross_out are never materialized.

Precision split: the score path (big GEMM + packed per-tile attention
matmuls + gate dot-products) runs in fp8 e4m3 with DoubleRow perf mode
(2 k-tiles of 128 per matmul at 0.5 cyc/row). On the value path the TAIL
pools stay fp16 (tpool lands raw in the output; fp8 there costs 3.5e-2
rel err) while the HEAD pools reuse the fp8 copy (hpool's quantization
noise washes through the Wsv^T projection; 1.35e-2 total, under the
2e-2 gate), which drops the fp16 head load entirely. The fp8 operands are
produced by a second SWDGE cast-load (f32->fp8) and transposed to
feature-major via the SBUF xbar with PAIRS of fp8 values packed in one
uint16 element; the resulting [feat-pair partition, 2, row] layout is
exactly DoubleRow's expected [K,2,N] k-tile shape (logical feature
f = 256c + 2p + i).  The folded A matrix is stored column-permuted
(per 256-block: even columns then odd columns) so the big GEMM's PSUM
partitions line up with the same pairing when its output chunks are used
as score-matmul weights.

Gate path: e = exp(scores/S) is transposed on the PE (identity matmul) so
den = e^T @ (-S*ones) and gs = e^T @ (head@u) become 1-column matmuls,
removing the partition-broadcast DRAM round-trip of hv entirely. The
sigmoid is evaluated as 1/(1+exp(-garg)) so the ACT engine only ever
needs the Exp table (Sigmoid lives in a different act-table set; each
switch would cost a 1.3us table reload); the sign flips ride host
constants (onesc=-S, mw=-S*mw, c0=-C0) and cancel in a_s/a_c. u and w2
are stored as fp8 hi+lo residual pairs - the gate dot-products were the
precision-dominant path (rel err 1.26e-2 -> 6.7e-3 on HW).
Scores carry a factor S=32 (folded into A) so the fp8 tensors sit inside
e4m3's dynamic range (max 240); exp applies scale 1/S.

Layout: batch is sharded 8 ways (512 batches/core); rows are processed in
tiles of 128 = 8 batches x 16 mentions. Cross-batch blocks are masked to
-inf via a rank-9 constant matmul and the per-tile key-mask via a rank-1
matmul, so softmax zeroes them exactly and the packed attention matrix is
block-diagonal -- which makes the pooling contractions plain matmuls
against block-diagonal weight columns built with a onehot multiply.
"""

import numpy as np
import ml_dtypes
import bass_rust
import concourse.bass as bass
import concourse.mybir as mybir
import concourse.tile as tile
from concourse.bass import ts
from concourse.bass_utils import run_bass_kernel_spmd

H = 768
B, M, N = 4096, 16, 16
NEG = -65504.0
P = 128
NCORES = 8
BC = B // NCORES          # batches per core = 512
ROWS = BC * M             # rows per core = 8192
TILES = ROWS // P         # 64 tiles (8 batches each)
ST = 4                    # tiles per supertile (GEMM moving N = 512)
NSUP = TILES // ST        # 16 supertiles
SN = ST * P               # 512 rows per supertile
GN = 512                  # GEMM moving width per PSUM pass (one bank)
KC = H // P               # 6 contraction chunks (128 each)
KC2 = H // (2 * P)        # 3 DoubleRow chunk pairs (256 each)
FO = 2 * H // P           # 12 score-feature chunks (A_s | A_c)
ACOLS = 2 * H + 1         # 1537: A_s | A_c | u
APAD = 1552               # ACOLS padded so the DoubleRow pair step is 16B-aligned
RPB = ROWS // M           # 512 pooled rows (batches) per core
S = 32.0                  # fp8 dynamic-range scale folded into A/u/w2/mw

F8 = mybir.dt.float8e4
F16 = mybir.dt.float16
F32 = mybir.dt.float32
U16 = mybir.dt.uint16
DR = mybir.MatmulPerfMode.DoubleRow
NP8 = ml_dtypes.float8_e4m3


def _split_sync_waits(nc):
    """Walrus caps sync waits per instruction (1 is the only universally
    accepted count in this toolchain). Hoist excess waits onto preceding
    single-wait EventSemaphore carriers on the same engine."""
    for f in nc.m.functions:
        for bb in f.blocks:
            il = bb.instructions
            new_il = []
            changed = False
            for inst in il:
                si = inst.sync_info
                if si is not None and len(si.on_wait) > 1:
                    waits = list(si.on_wait)
                    k = 0
                    while len(waits) > 1:
                        w, waits = waits[0], waits[1:]
                        d = bass_rust.InstEventSemaphore(
                            name=f"{inst.name}-wsplit{k}", ins=[], outs=[])
                        d.engine = inst.engine
                        d.sync_info = bass_rust.SyncInfo(on_wait=[w], on_update=[])
                        new_il.append(d)
                        k += 1
                        changed = True
                    inst.sync_info = bass_rust.SyncInfo(
                        on_wait=waits, on_update=list(si.on_update))
                new_il.append(inst)
            if changed:
                bb.instructions = new_il


def _build_nc(split=True):
    nc = bass.Bass(target_bir_lowering=False)

    head_d = nc.dram_tensor("head", [ROWS, H], F32, kind="ExternalInput")
    tail8_d = nc.dram_tensor("tail8", [ROWS, H], F8, kind="ExternalInput")
    taillo_d = nc.dram_tensor("taillo", [ROWS, H], F32, kind="ExternalInput")
    acat_d = nc.dram_tensor("acat", [KC2, P, 2, APAD], F8, kind="ExternalInput")
    w2_d = nc.dram_tensor("w2c", [KC2, P, 2, 2], F8, kind="ExternalInput")
    wsvT_d = nc.dram_tensor("wsvT", [KC, P, H], F16, kind="ExternalInput")
    c10_d = nc.dram_tensor("c10", [10, P], F16, kind="ExternalInput")
    onesc_d = nc.dram_tensor("onesc", [P, 1], F16, kind="ExternalInput")
    onehot_d = nc.dram_tensor("onehot", [P, ST * 8], F16, kind="ExternalInput")
    vs_d = nc.dram_tensor("vs", [10, TILES * P], F16, kind="ExternalInput")
    vc_d = nc.dram_tensor("vc", [10, TILES * P], F16, kind="ExternalInput")
    mw_d = nc.dram_tensor("mw", [P, TILES], F32, kind="ExternalInput")
    identh_d = nc.dram_tensor("identh", [P, P], F16, kind="ExternalInput")
    c0_d = nc.dram_tensor("c0", [P, 1], F32, kind="ExternalInput")
    out_d = nc.dram_tensor("out", [BC, H], F32, kind="ExternalOutput")

    with tile.TileContext(nc) as tc:
        _emit(nc, tc, head_d, tail8_d, taillo_d, acat_d, w2_d, wsvT_d, c10_d,
              onesc_d, onehot_d, vs_d, vc_d, mw_d,
              identh_d, c0_d, out_d)
    if split:
        _split_sync_waits(nc)
    return nc


def _emit(nc, tc, head_d, tail8_d, taillo_d, acat_d, w2_d, wsvT_d, c10_d,
          onesc_d, onehot_d, vs_d, vc_d, mw_d,
          identh_d, c0_d, out_d):
    from contextlib import ExitStack
    Exp = mybir.ActivationFunctionType.Exp
    Sig = mybir.ActivationFunctionType.Sigmoid
    Ident = mybir.ActivationFunctionType.Identity
    mult = mybir.AluOpType.mult
    ctx = ExitStack()
    with ctx:
        const = ctx.enter_context(tc.tile_pool(name="const", bufs=1))
        sup = ctx.enter_context(tc.tile_pool(name="sup", bufs=2))
        pt = ctx.enter_context(tc.tile_pool(name="pt", bufs=8))
        acc = ctx.enter_context(tc.tile_pool(name="acc", bufs=1))
        psg = ctx.enter_context(tc.tile_pool(name="psg", bufs=3, space="PSUM"))
        pss = ctx.enter_context(tc.tile_pool(name="pss", bufs=3, space="PSUM"))
        psw = ctx.enter_context(tc.tile_pool(name="psw", bufs=2, space="PSUM"))

        # ---- constants (emitted below, after the first transposes, so the
        # SP queue reaches the first-supertile transposes immediately; the
        # GEMM-blocking acat8 rides the startup-idle ACT queue) ----
        acat8 = const.tile([P, KC2, 2, APAD], F8)
        w2c8 = const.tile([P, KC2, 2, 2], F8)
        c10 = const.tile([10, P], F16)
        onesc = const.tile([P, 1], F16)
        onehot4 = const.tile([P, ST, 8], F16)
        vs_all = const.tile([10, TILES * P], F16)
        vc_all = const.tile([10, TILES * P], F16)

        # ---- per-core accumulators ----
        hp_all = acc.tile([P, KC, RPB], F16)   # pooled head, feature-major
        tp_all = acc.tile([P, KC, RPB], F16)   # pooled tail, feature-major

        # loads are batched per PAIR of supertiles: the SWDGE descriptor-gen
        # time on the Pool engine is ~1-2.4us per instruction regardless of
        # size, and 4 cast-loads/supertile made Pool the DMA-issue serializer
        SG = 2 * ST
        GSUP = SG // ST
        head_r = head_d.rearrange("(g t p) h -> g p t h", t=SG, p=P)
        tail8_r = tail8_d.rearrange("(g t p) h -> g p t h", t=SG, p=P)
        taillo_r = taillo_d.rearrange("(g t p) h -> g p t h", t=SG, p=P)

        loaded16 = {}
        loaded8 = {}

        def emit_loads16(g):
            # fp8 LO residual of tail (host-split): tail = t8 + t8lo to
            # ~2^-8 relative, which the TAIL value pools need (tpool lands
            # raw in the output; single fp8 there costs 3.5e-2 rel err).
            # The HEAD value pools read the fp8 h8 copy alone -- hpool's
            # quantization noise washes through the Wsv^T projection.
            t8lo = sup.tile([P, SG, H], F8, tag="t8lo", name=f"t8lo_{g}",
                            bufs=3)
            nc.gpsimd.dma_start(out=t8lo[:], in_=taillo_r[g])
            loaded16[g] = t8lo

        def emit_loads8(g):
            # fp8 copies feed the score-side GEMMs; h8 cast from the f32
            # rows by SWDGE, t8 pre-cast on host (so the lo residual pairs
            # exactly)
            h8 = sup.tile([P, SG, H], F8, tag="h8", name=f"h8_{g}", bufs=3)
            t8 = sup.tile([P, SG, H], F8, tag="t8", name=f"t8_{g}", bufs=3)
            nc.gpsimd.dma_start(out=h8[:], in_=head_r[g])
            nc.gpsimd.dma_start(out=t8[:], in_=tail8_r[g])
            loaded8[g] = (h8, t8)

        transposed = {}

        def emit_transpose(s):
            # xbar transpose to feature-major with fp8 PAIRS packed in uint16
            # -- ONE instruction per tensor per supertile. The whole-supertile
            # 2D transpose lands t-major: hT[p, t*KC2+c, n] = pair (2p,2p+1)
            # of feature chunk c, tile t, row n.
            h8, t8 = loaded8[s // GSUP]
            off = ST * (s % GSUP)
            hT = sup.tile([P, ST * KC2, P], U16, tag="hT", name=f"hT{s}", bufs=3)
            tT = sup.tile([P, ST * KC2, P], U16, tag="tT", name=f"tT{s}", bufs=3)
            nc.sync.dma_start_transpose(
                hT[:], h8[:, off:off + ST, :].bitcast(U16).rearrange(
                    "p t m -> p (t m)"))
            nc.sync.dma_start_transpose(
                tT[:], t8[:, off:off + ST, :].bitcast(U16).rearrange(
                    "p t m -> p (t m)"))
            transposed[s] = (hT, tT)

        def f8v(tT_, t, cc):
            # DoubleRow moving view of one tile's pair-packed chunk:
            # [K=128, 2, n=128]
            return tT_[:, t * KC2 + cc, :].bitcast(F8).rearrange(
                "p (n two) -> p two n", two=2)

        def f8g(tT_, cc):
            # DoubleRow moving view of chunk cc across the whole supertile:
            # [K=128, 2, t, n] (4D AP; rows grouped per tile)
            return tT_.rearrange("p (t c) n -> p t c n", c=KC2)[
                :, :, cc, :].bitcast(F8).rearrange(
                "p t (n two) -> p two t n", two=2)

        out_sb = acc.tile([P, BC // P, H], F32)
        out_r = out_d.rearrange("(r p) h -> p r h", p=P)

        def emit_final(r, piece):
            # out[batch, :] = hpool @ Wsv^T + tpool, computed ROW-major:
            # hp_all chunks are the (free) stationary with batches on the
            # free axis, wsvT moving; tpool is transpose-accumulated into
            # the same PSUM group via an identity moving operand. One batched
            # ACT copy per PSUM bank evacuates, then DMA-store. Emitted in
            # per-128-batch halves spread across the supertile stream so the
            # psg rotation and the in-order PE queue are never held long.
            js = range(0, 4) if piece == 0 else range(4, KC)
            pX = psg.tile([P, GN], F32, tag="pg", name=f"pf{r}_{piece}")
            for j in js:
                tgt = pX[:, ts(j - js.start, P)]
                for c in range(KC):
                    nc.tensor.matmul(tgt, hp_all[:, c, ts(r, P)],
                                     wsvT[:, c, ts(j, P)],
                                     start=(c == 0), stop=False)
                nc.tensor.matmul(tgt, tp_all[:, j, ts(r, P)], identh[:],
                                 start=False, stop=True)
            if piece == 0:
                nc.scalar.copy(out_sb[:, r, 0:GN], pX[:])
            else:
                nc.scalar.copy(out_sb[:, r, GN:H], pX[:, :H - GN])
                nc.sync.dma_start(out=out_r[:, r, :], in_=out_sb[:, r, :])

        emit_loads8(0)
        emit_loads16(0)
        emit_transpose(0)
        # constants follow the supertile-0 transposes on SP so nothing
        # delays the first GEMM's inputs
        nc.sync.dma_start(out=acat8[:],
                          in_=acat_d.rearrange("c p i m -> p c i m"))
        nc.sync.dma_start(out=w2c8[:],
                          in_=w2_d.rearrange("c p i m -> p c i m"))
        nc.sync.dma_start(out=c10[:], in_=c10_d[:, :])
        nc.sync.dma_start(out=onesc[:], in_=onesc_d[:, :])
        nc.sync.dma_start(out=onehot4[:], in_=onehot_d.rearrange(
            "p (t e) -> p t e", e=8))
        nc.sync.dma_start(out=vs_all[:], in_=vs_d[:, :])
        nc.sync.dma_start(out=vc_all[:], in_=vc_d[:, :])
        wsvT = const.tile([P, KC, H], F16)
        nc.sync.dma_start(out=wsvT[:], in_=wsvT_d.rearrange("c p m -> p c m"))
        mw_all = const.tile([P, TILES], F32)
        nc.sync.dma_start(out=mw_all[:], in_=mw_d[:, :])
        identh = const.tile([P, P], F16)
        nc.sync.dma_start(out=identh[:], in_=identh_d[:, :])
        c0 = const.tile([P, 1], F32)
        nc.sync.dma_start(out=c0[:], in_=c0_d[:, :])
        hA8s = {}

        def emit_gemm(s):
            # -- big GEMM: hA = head @ [A_s | A_c], feature-major, fp8 DR --
            hT, tT = transposed[s]
            hA8 = sup.tile([P, FO, SN], F8, tag="hA8", name=f"hA8_{s}")
            for j in range(FO):
                for hh in range(SN // GN):
                    pg = psg.tile([P, GN], F32, tag="pg")
                    for tt in range(ST):
                        for cc in range(KC2):
                            nc.tensor.matmul(pg[:, ts(tt, P)],
                                             acat8[:, cc, :, ts(j, P)],
                                             f8v(hT, tt, cc),
                                             start=(cc == 0),
                                             stop=(cc == KC2 - 1),
                                             perf_mode=DR)
                    if j % 2 == 0 or j == 11:
                        nc.scalar.copy(hA8[:, j, ts(hh, GN)], pg[:])
                    else:
                        nc.vector.tensor_copy(hA8[:, j, ts(hh, GN)], pg[:])
            hA8s[s] = hA8

        def emit_tiles(s_idx):
            t8log = loaded16[s_idx // GSUP]
            h8g, t8g = loaded8[s_idx // GSUP]
            voff = ST * (s_idx % GSUP)
            hT, tT = transposed.pop(s_idx)
            hA8 = hA8s.pop(s_idx)
            h16 = h8g[:, voff:voff + ST, :]
            t16 = t8g[:, voff:voff + ST, :]
            t16lo = t8log[:, voff:voff + ST, :]

            # One quad = the 4 tiles of this supertile. All [P,1]-sized gate
            # and copy work is batched across the quad to amortize per-inst
            # overhead; wp4 strides each tile's workspace by 256 f32 so no
            # matmul output crosses a PSUM bank boundary.
            # wp4[:, t, :] cols: 0:48 ps_hp, 48:96 ps_tp, 96 ws_s, 97 ws_c,
            # 98 gs, 99 gc, 100 S*den_s, 101 S*den_c, 102 hv, 103 tv,
            # 104:232 e^T (fp16 x256 via bitcast), 232:256 pad.
            tg0 = s_idx * ST
            wp4 = psw.tile([P, ST, 128], F32, tag="wp")
            ps2s = [pss.tile([P, 2, 2, P], F32, tag="ps", name=f"ps{s_idx}_{h}")
                    for h in range(2)]
            eT4 = pss.tile([P, ST, 2, P], F16, tag="ps", name=f"eT{s_idx}")

            for t in range(ST):
                tg = tg0 + t
                # -- packed scores (8 batches x 16x16) + masks (one rank-10
                # matmul: rows 0-8 cross-batch block mask, row 9 key mask) --
                ps_pair = ps2s[t // 2][:, t % 2]
                ps_s = ps_pair[:, 0, :]
                ps_c = ps_pair[:, 1, :]
                for cc in range(KC2):
                    nc.tensor.matmul(ps_s, hA8[:, 2 * cc:2 * cc + 2, ts(t, P)],
                                     f8v(hT, t, cc),
                                     start=(cc == 0), stop=False, perf_mode=DR)
                nc.tensor.matmul(ps_s, c10[:], vs_all[:, ts(tg, P)],
                                 start=False, stop=True)
                for cc in range(KC2):
                    nc.tensor.matmul(ps_c,
                                     hA8[:, KC + 2 * cc:KC + 2 * cc + 2, ts(t, P)],
                                     f8v(tT, t, cc),
                                     start=(cc == 0), stop=False, perf_mode=DR)
                nc.tensor.matmul(ps_c, c10[:], vc_all[:, ts(tg, P)],
                                 start=False, stop=True)

                # -- gate dot inputs: hv = head@u, tv = tail@w2 (key-major,
                # 1-column DoubleRow matmuls, ~free on the PE); u and w2 are
                # fp8 hi+lo pairs (the gate is the precision-dominant path) --
                for k in range(4 * KC2):
                    cc, i, r = k // 4, (k // 2) % 2, k % 2
                    nc.tensor.matmul(wp4[:, t, 102:103],
                                     f8v(hT, t, cc)[:, i, :],
                                     acat8[:, cc, i, 1536 + r:1537 + r],
                                     start=(k == 0), stop=(k == 4 * KC2 - 1))
                for k in range(4 * KC2):
                    cc, i, r = k // 4, (k // 2) % 2, k % 2
                    nc.tensor.matmul(wp4[:, t, 103:104],
                                     f8v(tT, t, cc)[:, i, :],
                                     w2c8[:, cc, i, r:r + 1],
                                     start=(k == 0), stop=(k == 4 * KC2 - 1))

            # -- softmax numerators (free axis), one exp per 2 tiles;
            # masked lanes are ~-2e3 after the 1/S exp scale --
            e2s = []
            for h in range(2):
                e2 = pt.tile([P, 2, 2, P], F16, tag=f"e2_{h}", bufs=3)
                nc.scalar.activation(out=e2[:], in_=ps2s[h][:], func=Exp,
                                     bias=0.0, scale=1.0 / S)
                e2s.append(e2)

            # -- e^T on the PE so den/gs become 1-column matmuls --
            for t in range(ST):
                e_pair = e2s[t // 2][:, t % 2]
                nc.tensor.transpose(eT4[:, t, 0, :], e_pair[:, 0, :], identh[:])
                nc.tensor.transpose(eT4[:, t, 1, :], e_pair[:, 1, :], identh[:])

            # -- batched PSUM->SBUF staging for the whole quad --
            esT4 = pt.tile([P, ST, 2, P], F16, tag="esT", bufs=2)
            nc.vector.tensor_copy(esT4[:], eT4[:])
            hvtv4 = pt.tile([P, ST, 2], F16, tag="hvtv", bufs=2)
            nc.vector.tensor_copy(hvtv4[:], wp4[:, :, 102:104])

            # cols: 98 gs_num, 99 gc_num, 100 S*den_s, 101 S*den_c
            for t in range(ST):
                nc.tensor.matmul(wp4[:, t, 98:99], esT4[:, t, 0, :],
                                 hvtv4[:, t, 0:1], start=True, stop=True)
                nc.tensor.matmul(wp4[:, t, 99:100], esT4[:, t, 1, :],
                                 hvtv4[:, t, 1:2], start=True, stop=True)
                nc.tensor.matmul(wp4[:, t, 100:101], esT4[:, t, 0, :],
                                 onesc[:], start=True, stop=True)
                nc.tensor.matmul(wp4[:, t, 101:102], esT4[:, t, 1, :],
                                 onesc[:], start=True, stop=True)

            # -- batched gate math (gate-as-exp; sign flips ride the host
            # constants onesc=-S, mw=-S*mw, c0=-C0 so the ACT engine only
            # ever needs the Exp table) --
            rden4 = pt.tile([P, ST, 2], F32, tag="rden", bufs=2)
            nc.vector.reciprocal(out=rden4[:], in_=wp4[:, :, 100:102])
            m4 = pt.tile([P, ST, 2], F32, tag="m4", bufs=2)
            nc.vector.tensor_mul(out=m4[:], in0=wp4[:, :, 98:100], in1=rden4[:])
            garg4 = pt.tile([P, ST], F32, tag="garg", bufs=2)
            nc.vector.tensor_add(out=garg4[:], in0=m4[:, :, 0], in1=m4[:, :, 1])
            eg4 = pt.tile([P, ST], F32, tag="eg", bufs=2)
            nc.scalar.activation(out=eg4[:], in_=garg4[:], func=Exp,
                                 bias=c0[:, 0:1], scale=1.0)
            gp4 = pt.tile([P, ST], F32, tag="gp", bufs=2)
            nc.vector.tensor_scalar_add(out=gp4[:], in0=eg4[:], scalar1=1.0)
            gate4 = pt.tile([P, ST], F32, tag="gate", bufs=2)
            nc.vector.reciprocal(out=gate4[:], in_=gp4[:])

            # -- pooling coefficient vectors (fold S*mw and 1/(S den)) --
            mw4 = mw_all[:, tg0:tg0 + ST]
            mwg4 = pt.tile([P, ST], F16, tag="mwg", bufs=2)   # S*mw*gate
            nc.vector.tensor_mul(out=mwg4[:], in0=mw4, in1=gate4[:])
            a_s4 = pt.tile([P, ST], F16, tag="a_s", bufs=2)
            nc.vector.tensor_mul(out=a_s4[:], in0=mwg4[:], in1=rden4[:, :, 0])
            mwc4 = pt.tile([P, ST], F16, tag="mwc", bufs=2)   # S*mw*(1-gate)
            nc.vector.tensor_sub(out=mwc4[:], in0=mw4, in1=mwg4[:])
            a_c4 = pt.tile([P, ST], F16, tag="a_c", bufs=2)
            nc.vector.tensor_mul(out=a_c4[:], in0=mwc4[:], in1=rden4[:, :, 1])

            # -- ws = e^T @ a : per-key pooled weights (block-diag safe) --
            for t in range(ST):
                e_pair = e2s[t // 2][:, t % 2]
                nc.tensor.matmul(wp4[:, t, 96:97], e_pair[:, 0, :],
                                 a_s4[:, t:t + 1], start=True, stop=True)
                nc.tensor.matmul(wp4[:, t, 97:98], e_pair[:, 1, :],
                                 a_c4[:, t:t + 1], start=True, stop=True)

            # -- block-diagonal weight columns via onehot, whole quad --
            diag_s4 = pt.tile([P, ST, 8], F16, tag="diag_s", bufs=2)
            diag_c4 = pt.tile([P, ST, 8], F16, tag="diag_c", bufs=2)
            nc.vector.tensor_tensor(out=diag_s4[:],
                                    in0=wp4[:, :, 96:97].to_broadcast([P, ST, 8]),
                                    in1=onehot4[:], op=mult)
            nc.vector.tensor_tensor(out=diag_c4[:],
                                    in0=wp4[:, :, 97:98].to_broadcast([P, ST, 8]),
                                    in1=onehot4[:], op=mult)

            # -- pools: feature-major pooled vectors, 8 batches per tile --
            for t in range(ST):
                ps_hp = wp4[:, t, 0:48].rearrange("p (c e) -> p c e", e=8)
                ps_tp = wp4[:, t, 48:96].rearrange("p (c e) -> p c e", e=8)
                for c in range(KC):
                    nc.tensor.matmul(ps_hp[:, c, :], h16[:, t, ts(c, P)],
                                     diag_s4[:, t, :], start=True, stop=True)
                    nc.tensor.matmul(ps_tp[:, c, :], t16[:, t, ts(c, P)],
                                     diag_c4[:, t, :], start=True, stop=False)
                    nc.tensor.matmul(ps_tp[:, c, :], t16lo[:, t, ts(c, P)],
                                     diag_c4[:, t, :], start=False, stop=True)
            nc.vector.tensor_copy(
                hp_all[:, :, tg0 * 8:(tg0 + ST) * 8].rearrange(
                    "p c (t e) -> p t c e", e=8),
                wp4[:, :, 0:48].rearrange("p t (c e) -> p t c e", e=8))
            nc.vector.tensor_copy(
                tp_all[:, :, tg0 * 8:(tg0 + ST) * 8].rearrange(
                    "p c (t e) -> p t c e", e=8),
                wp4[:, :, 48:96].rearrange("p t (c e) -> p t c e", e=8))

        for s_idx in range(NSUP):
            if 8 <= s_idx <= 13:
                emit_final((s_idx - 8) // 2, (s_idx - 8) % 2)
            if s_idx % GSUP == 0 and s_idx // GSUP + 1 < NSUP // GSUP:
                emit_loads8(s_idx // GSUP + 1)
                emit_loads16(s_idx // GSUP + 1)
            if s_idx + 1 < NSUP:
                emit_transpose(s_idx + 1)
            emit_gemm(s_idx)
            emit_tiles(s_idx)

        emit_final(3, 0)
        emit_final(3, 1)


_NC_CACHE = None


def _get_nc():
    global _NC_CACHE
    if _NC_CACHE is None:
        _NC_CACHE = _build_nc()
    return _NC_CACHE


def _host_prep(Wsq, Wsk, Wsv, Wcq, Wck, Wg, bg, bsv,
               head_mask, tail_mask):
    """Fold weights; build per-core constant tensors (shared across cores
    except the mask-derived ones)."""
    f64 = np.float64
    scale = 1.0 / np.sqrt(f64(H))
    A_s = (Wsq.astype(f64).T @ Wsk.astype(f64)) * scale
    A_c = (Wcq.astype(f64).T @ Wck.astype(f64)) * scale
    A = np.concatenate([A_s, A_c], axis=1)                         # [768, 1536]
    # per 256-block of output features: even columns then odd columns, so
    # the big GEMM's PSUM chunk pairs (2c, 2c+1) hold features 256c+2p+i
    colperm = np.concatenate([
        np.concatenate([np.arange(256 * b, 256 * b + 256, 2),
                        np.arange(256 * b + 1, 256 * b + 256, 2)])
        for b in range(2 * H // 256)])
    Wg1 = Wg[0, :H].astype(f64)
    w2 = Wg[0, H:].astype(f64)
    u = Wsv.astype(f64).T @ Wg1
    uS = S * u
    u_hi = (uS.astype(np.float32)).astype(NP8)
    u_lo = uS - u_hi.astype(f64)                  # quantized again by the cast
    w2S = S * w2
    w2_hi = (w2S.astype(np.float32)).astype(NP8)
    w2_lo = w2S - w2_hi.astype(f64)
    acat = np.concatenate([A[:, colperm], u_hi.astype(f64)[:, None] / S,
                           u_lo[:, None] / S,
                           np.zeros((H, APAD - ACOLS - 1))], axis=1)  # [768, 1552]
    # rows (input features) interleaved: acat8[c, p, i] = S*acat[256c+2p+i]
    acat8 = (S * acat).reshape(KC2, P, 2, APAD).astype(NP8)
    w2_8 = np.stack([w2_hi.astype(f64), w2_lo], axis=-1)
    w2_8 = (w2_8).reshape(KC2, P, 2, 2).astype(NP8)
    wsvT_t = Wsv.astype(f64).T.reshape(KC, P, H).astype(np.float16)

    g = np.arange(P) // M                                          # group id per row
    # rank-10 combined mask operand: rows 0-8 = cross-batch block mask
    # (NEG everywhere, un-NEG within own 16-row block), row 9 = key mask
    c10 = np.zeros((10, P), np.float16)
    c9r = np.zeros((9, P), np.float16)
    c10[0] = 1.0
    c9r[0] = NEG
    for k in range(8):
        c10[1 + k] = (g == k).astype(np.float16)
        c9r[1 + k] = -NEG * (g == k).astype(np.float16)
    c10[9] = 1.0
    onesc = np.full((P, 1), -S, np.float16)   # negative: see gate-as-exp note
    onehot = np.zeros((P, 8), np.float16)
    onehot[np.arange(P), g] = 1.0
    onehot = np.tile(onehot, (1, 4))          # one copy per quad tile

    C0 = float(bg[0] + f64(bsv) @ Wg1)
    c0 = np.full((P, 1), -C0, np.float32)     # negated: gate-as-exp
    identh = np.eye(P, dtype=np.float16)

    # per-core mask-derived tensors: [10, TILES*P] moving operands whose
    # rows 0-8 repeat c9r every tile and row 9 carries the key mask
    hm = head_mask.reshape(NCORES, BC, M)
    tm = tail_mask.reshape(NCORES, BC, N)
    c9r_t = np.tile(c9r[:, None, :], (1, TILES, 1)).reshape(9, TILES * P)
    vs, vc, mw = [], [], []
    for i in range(NCORES):
        vsi = ((1 - hm[i]).astype(np.float16) * np.float16(NEG)
               ).reshape(1, TILES * P)
        vci = ((1 - tm[i]).astype(np.float16) * np.float16(NEG)
               ).reshape(1, TILES * P)
        vs.append(np.concatenate([c9r_t, vsi], axis=0))
        vc.append(np.concatenate([c9r_t, vci], axis=0))
        e = np.exp(hm[i].astype(f64))
        mwi = (-S * e / e.sum(axis=1, keepdims=True)).astype(np.float32)
        mw.append(mwi.reshape(TILES, P).T.copy())                    # [P, TILES]
    shared = dict(acat=acat8, w2c=w2_8, wsvT=wsvT_t, c10=c10,
                  onesc=onesc, onehot=onehot,
                  identh=identh, c0=c0)
    return shared, vs, vc, mw


def _core_feeds(head_mentions, tail_mentions, shared, vs, vc, mw, i):
    hm = head_mentions.reshape(NCORES, ROWS, H)
    tm = tail_mentions.reshape(NCORES, ROWS, H)
    t = np.ascontiguousarray(tm[i])
    t8 = t.astype(NP8)                       # hi fp8 (host cast, exact pair)
    tlo = t - t8.astype(np.float32)          # residual, SWDGE-cast to fp8
    feeds = {"head": np.ascontiguousarray(hm[i]),
             "tail8": t8, "taillo": tlo,
             "vs": vs[i], "vc": vc[i], "mw": mw[i]}
    feeds.update(shared)
    return feeds


def _reference_numpy(head_mentions, tail_mentions, head_mask, tail_mask,
                     Wsq, bsq, Wsk, bsk, Wsv, bsv, Wcq, bcq, Wck, bck, Wg, bg):
    """Exact fallback (only used if projection biases are nonzero)."""
    f = np.float32
    scale = f(1.0) / np.sqrt(f(H))
    hm = head_mentions.astype(f)
    tm = tail_mentions.astype(f)
    sq = hm @ Wsq.T + bsq
    sk = hm @ Wsk.T + bsk
    sv = hm @ Wsv.T + bsv
    ss = np.einsum("bmh,bnh->bmn", sq, sk) * scale
    ss = np.where(head_mask[:, None, :] == 0, f(NEG), ss)
    ss = ss - ss.max(-1, keepdims=True)
    e = np.exp(ss)
    sw = e / e.sum(-1, keepdims=True)
    self_out = np.einsum("bmn,bnh->bmh", sw, sv)
    cq = hm @ Wcq.T + bcq
    ck = tm @ Wck.T + bck
    cs = np.einsum("bmh,bnh->bmn", cq, ck) * scale
    cs = np.where(tail_mask[:, None, :] == 0, f(NEG), cs)
    cs = cs - cs.max(-1, keepdims=True)
    ec = np.exp(cs)
    cw = ec / ec.sum(-1, keepdims=True)
    cross_out = np.einsum("bmn,bnh->bmh", cw, tm)
    gate_in = np.concatenate([self_out, cross_out], axis=-1)
    gate = 1.0 / (1.0 + np.exp(-(np.einsum("bmh,oh->bmo", gate_in, Wg) + bg)))
    fused = gate * self_out + (1 - gate) * cross_out
    mexp = np.exp(head_mask.astype(f))
    mw = (mexp / mexp.sum(1, keepdims=True))[:, :, None]
    return (fused * mw).sum(axis=1)


def kernel(head_mentions, tail_mentions, head_mask, tail_mask,
           Wsq, bsq, Wsk, bsk, Wsv, bsv, Wcq, bcq, Wck, bck, Wg, bg,
           _trace=False):
    head_mentions = np.asarray(head_mentions)
    tail_mentions = np.asarray(tail_mentions)
    head_mask = np.asarray(head_mask)
    tail_mask = np.asarray(tail_mask)
    args = dict(Wsq=np.asarray(Wsq), bsq=np.asarray(bsq), Wsk=np.asarray(Wsk),
                bsk=np.asarray(bsk), Wsv=np.asarray(Wsv), bsv=np.asarray(bsv),
                Wcq=np.asarray(Wcq), bcq=np.asarray(bcq), Wck=np.asarray(Wck),
                bck=np.asarray(bck), Wg=np.asarray(Wg), bg=np.asarray(bg))

    # The folded formulation absorbs bg/bsv exactly; nonzero Q/K-side biases
    # (never produced by this problem's setup) would change the softmax and
    # are handled by the exact numpy fallback.
    if any(np.any(args[k] != 0) for k in ("bsq", "bsk", "bcq", "bck")):
        return _reference_numpy(head_mentions, tail_mentions, head_mask,
                                tail_mask, **args).astype(np.float32)

    shared, vs, vc, mw = _host_prep(args["Wsq"], args["Wsk"], args["Wsv"],
                                    args["Wcq"], args["Wck"], args["Wg"],
                                    args["bg"], args["bsv"],
                                    head_mask, tail_mask)

    nc = _get_nc()
    in_maps = [_core_feeds(head_mentions, tail_mentions, shared, vs, vc, mw, i)
               for i in range(NCORES)]
    res = run_bass_kernel_spmd(nc, in_maps, core_ids=list(range(NCORES)),
                               trace=_trace)
    out = np.concatenate([res.results[i]["out"] for i in range(NCORES)], axis=0)
    if _trace:
        kernel._last_result = res
    return out.astype(np.float32)

